# revision 1
# baseline (speedup 1.0000x reference)
"""Mamba block kernel for Trainium2 (8 NeuronCores).

Sharding: batch (2-way) x tensor-parallel over d_inner (4-way).
Core c handles batch c//4 and d_inner channels [(c%4)*512, (c%4+1)*512).
Weights are pre-transposed/sliced on the host; the 4 TP partial outputs
per batch are summed on the host (no on-chip reduction of the output).

Device pipeline per core (one NEFF, phases overlap via Tile scheduling):
  A. res-add + RMSNorm in row layout (ACT square-accumulate for the
     variance), PE-transpose via diag matmuls to h^T [d_model, L] bf16
  B. in_proj x-half (bf16 matmuls, f32 PSUM accumulate, t-major) with the
     causal depthwise conv (4 scalar_tensor_tensor taps + fused SiLU) and
     the x_proj partial matmuls interleaved per time chunk
  D. AllReduce of x_dbl partials (groups [[0-3],[4-7]], DRAM bounce
     buffers); the z-half of in_proj + SiLU runs under the collective's
     latency
  F. per d-chunk (software-pipelined across chunks):
       dt = softplus(dt_proj + bias) as exp on ACT + 4-term log1p series
       on DVE (exact to ~4e-6 for this dt range), written as f32r;
       selective scan over 16 expanded tiles [128=(8 d x 16 n), L]:
         a = exp(dt*A):   PE f32r replicate-matmul -> PSUM, ACT Exp with
                          per-partition A scale
         b = u*B:         u replicated through a DRAM scratch (free-form
                          source APs, 4 subtiles per DMA), times a B
                          broadcast tile, bf16 2x on DVE
         h:               tensor_tensor_scan (mult/add) over full L
         y = sum_n C*h:   C-multiply on DVE then 0/1-selection matmuls
                          accumulating all 16 subtiles into one PSUM tile
       skip (D*x) + gate (silu(z)) -> yg bf16
  G. out_proj partial (bf16) -> [L, 1024] f32 -> DRAM

All inter-engine broadcast/replication goes through PE 0/1-matmuls or
DMA with free-form DRAM APs; no partition-broadcast instructions.
"""

import sys

sys.path.insert(0, "/opt/trn_rl_repo")

import numpy as np

import concourse.bacc as bacc
import concourse.tile as tile
from concourse import mybir
from concourse.bass_utils import run_bass_kernel_spmd

F32 = mybir.dt.float32
F32R = mybir.dt.float32r
BF16 = mybir.dt.bfloat16
AF = mybir.ActivationFunctionType
OP = mybir.AluOpType

D_MODEL = 1024
D_INNER = 2048
NST = 16          # d_state
DT_RANK = 64
DCONV = 4
BATCH = 2
L = 2048
EPS = 1e-5

N_CORES = 8
TPG = 4                    # tensor-parallel group size
DLOC = D_INNER // TPG      # 512 channels per core
DC = DLOC // 128           # 4 partition chunks of x-channels
KC = D_MODEL // 128        # 8 contraction chunks
TCH = L // 512             # 4 time chunks of 512
RT = L // 128              # 16 row tiles
NSUB = 128 // NST          # 8 d-channels per expanded tile
SPC = 128 // NSUB          # 16 subtiles per d-chunk


def _build():
    nc = bacc.Bacc("TRN2", target_bir_lowering=False, debug=False,
                   enable_asserts=True, num_devices=N_CORES)

    def din(name, shape, dt=F32):
        return nc.dram_tensor(name, shape, dt, kind="ExternalInput").ap()

    hid = din("hid", [L, D_MODEL])
    res = din("res", [L, D_MODEL])
    winx = din("winx", [D_MODEL, DLOC], BF16)   # in_proj_w[x-slice].T
    winz = din("winz", [D_MODEL, DLOC], BF16)   # in_proj_w[z-slice].T
    wxT = din("wxT", [DLOC, 96], BF16)          # x_proj_w[:, slice].T
    wdtT = din("wdtT", [DT_RANK, DLOC], BF16)   # dt_proj_w[slice].T
    woutT = din("woutT", [DLOC, D_MODEL], BF16)  # out_proj_w[:, slice].T
    convw = din("convw", [128, DC * DCONV])     # [p, dc*4+k]
    convb = din("convb", [128, DC])
    dtb = din("dtb", [128, DC])
    dpar = din("dpar", [128, DC])
    a_sc = din("a_sc", [128, DC * SPC])         # per-tile A scale column
    normw = din("normw", [128, KC])
    selm = din("selm", [128, SPC * 128], BF16)  # 16 selection matrices
    expm = din("expm", [128, SPC * 128], F32R)  # 16 expansion matrices
    ident = din("ident", [128, 128])

    out_part = nc.dram_tensor("out_part", [L, D_MODEL], F32,
                              kind="ExternalOutput").ap()

    with tile.TileContext(nc) as tc:
        # Pools with overlapping lifetimes, managed manually:
        #   cst: whole kernel (small constants + weights)
        #   pW:  phase A..B (winx/winz, hT)
        #   pBC: phase B..F (zg, xb)
        #   pDE: phase D..F (dt, ub, bbc, cbc)
        #   pY:  phase F..G (yg)
        cst = tc.alloc_tile_pool(name="cst", bufs=1)
        dram = tc.alloc_tile_pool(name="dram", bufs=1, space="DRAM")
        pW = tc.alloc_tile_pool(name="pW", bufs=1)

        # ---- constants / weights to SBUF ----
        conv_sb = cst.tile([128, DC * DCONV], F32)
        nc.sync.dma_start(conv_sb[:], convw[:])
        convb_sb = cst.tile([128, DC], F32)
        nc.sync.dma_start(convb_sb[:], convb[:])
        dtb_sb = cst.tile([128, DC], F32)
        nc.sync.dma_start(dtb_sb[:], dtb[:])
        dpar_sb = cst.tile([128, DC], F32)
        nc.sync.dma_start(dpar_sb[:], dpar[:])
        asc_sb = cst.tile([128, DC * SPC], F32)
        nc.sync.dma_start(asc_sb[:], a_sc[:])
        normw_sb = cst.tile([128, KC], F32)
        nc.sync.dma_start(normw_sb[:], normw[:])
        sel_sb = cst.tile([128, SPC * 128], BF16)
        nc.sync.dma_start(sel_sb[:], selm[:])
        exp_sb = cst.tile([128, SPC * 128], F32R)
        nc.sync.dma_start(exp_sb[:], expm[:])
        id_sb = cst.tile([128, 128], F32)
        nc.sync.dma_start(id_sb[:], ident[:])
        eps_sb = cst.tile([128, 1], F32)
        nc.vector.memset(eps_sb[:], EPS)
        wx_sb = [cst.tile([128, 96], BF16, tag=f"wx{d}", name=f"wx{d}")
                 for d in range(DC)]
        for d in range(DC):
            nc.sync.dma_start(wx_sb[d][:], wxT[128 * d:128 * (d + 1), :])
        wdt_sb = cst.tile([DT_RANK, DLOC], BF16)
        nc.sync.dma_start(wdt_sb[:], wdtT[:])
        wout_sb = [cst.tile([128, D_MODEL], BF16, tag=f"wo{d}", name=f"wo{d}")
                   for d in range(DC)]
        for d in range(DC):
            nc.sync.dma_start(wout_sb[d][:], woutT[128 * d:128 * (d + 1), :])
        winx_sb = [pW.tile([128, DLOC], BF16, tag=f"winx{k}", name=f"winx{k}")
                   for k in range(KC)]
        winz_sb = [pW.tile([128, DLOC], BF16, tag=f"winz{k}", name=f"winz{k}")
                   for k in range(KC)]
        hT = [pW.tile([128, L], BF16, tag=f"hT{k}", name=f"hT{k}")
              for k in range(KC)]

        # ====== Phase A: res add + RMSNorm + transpose ======
        ps_mm = tc.alloc_tile_pool(name="ps_mm", bufs=4, space="PSUM")
        with tc.tile_pool(name="pA", bufs=2) as pA, \
             tc.tile_pool(name="pA2", bufs=2) as pA2:
            for rt in range(RT):
                t0 = 128 * rt
                ld1 = pA.tile([128, D_MODEL], F32, tag="ld1")
                nc.sync.dma_start(ld1[:], hid[t0:t0 + 128, :])
                ld2 = pA.tile([128, D_MODEL], F32, tag="ld2")
                nc.sync.dma_start(ld2[:], res[t0:t0 + 128, :])
                r = pA.tile([128, D_MODEL], F32, tag="r")
                nc.vector.tensor_add(r[:], ld1[:], ld2[:])
                sq = pA2.tile([128, D_MODEL], F32, tag="sq", bufs=1)
                st = pA2.tile([128, 1], F32, tag="st")
                nc.scalar.activation(sq[:], r[:], AF.Square, accum_out=st[:])
                sg = pA2.tile([128, 1], F32, tag="sg")
                nc.scalar.activation(sg[:], st[:], AF.Sqrt,
                                     bias=eps_sb[:], scale=1.0 / D_MODEL)
                rstd = pA2.tile([128, 1], F32, tag="rstd")
                nc.vector.reciprocal(rstd[:], sg[:])
                hrow = pA2.tile([128, D_MODEL], F32, tag="hrow")
                nc.vector.tensor_scalar_mul(hrow[:], r[:], rstd[:])
                for k in range(KC):
                    pt = ps_mm.tile([128, 512], F32, tag="pm")
                    nc.tensor.transpose(pt[:, 0:128],
                                        hrow[:, 128 * k:128 * (k + 1)],
                                        id_sb[:])
                    nc.scalar.activation(hT[k][:, t0:t0 + 128], pt[:, 0:128],
                                         AF.Copy)

        for k in range(KC):
            nc.sync.dma_start(winx_sb[k][:], winx[128 * k:128 * (k + 1), :])
            nc.sync.dma_start(winz_sb[k][:], winz[128 * k:128 * (k + 1), :])

        # ====== Phase B: in_proj (t-major) + conv + x_proj ======
        pBC = tc.alloc_tile_pool(name="pBC", bufs=1, side="right")
        zg = [pBC.tile([128, L], BF16, tag=f"zg{d}", name=f"zg{d}")
              for d in range(DC)]
        xb = [pBC.tile([128, L], BF16, tag=f"xb{d}", name=f"xb{d}")
              for d in range(DC)]
        xdbl_p = pBC.tile([96, L], F32)
        pX = tc.alloc_tile_pool(name="pX", bufs=1, side="right")
        xpad = [pX.tile([128, L + DCONV - 1], BF16, tag=f"xpad{d}",
                        name=f"xpad{d}") for d in range(DC)]
        for d in range(DC):
            nc.vector.memset(xpad[d][:, 0:DCONV - 1], 0.0)
        with tc.tile_pool(name="pC", bufs=3) as pC:
            def emit_conv(d, t):
                o = 512 * t
                acc = pC.tile([128, 512], BF16, tag="acc", name="acc")
                nc.vector.tensor_scalar_mul(
                    acc[:], xpad[d][:, o:o + 512],
                    conv_sb[:, d * DCONV:d * DCONV + 1])
                for k in range(1, DCONV):
                    nc.vector.scalar_tensor_tensor(
                        acc[:], xpad[d][:, o + k:o + k + 512],
                        conv_sb[:, d * DCONV + k:d * DCONV + k + 1],
                        acc[:], OP.mult, OP.add)
                nc.scalar.activation(xb[d][:, o:o + 512], acc[:], AF.Silu,
                                     bias=convb_sb[:, d:d + 1])

            for t in range(TCH):
                for d in range(DC):   # x blocks first
                    pm = ps_mm.tile([128, 512], F32, tag="pm")
                    for k in range(KC):
                        nc.tensor.matmul(pm[:],
                                         winx_sb[k][:, 128 * d:128 * (d + 1)],
                                         hT[k][:, 512 * t:512 * (t + 1)],
                                         start=(k == 0), stop=(k == KC - 1))
                    o0 = DCONV - 1 + 512 * t
                    nc.scalar.activation(xpad[d][:, o0:o0 + 512], pm[:],
                                         AF.Copy)
                    if t > 0:
                        emit_conv(d, t - 1)
                    if t == TCH - 1:
                        emit_conv(d, t)
                # x_proj for finished chunks: chunk t-1 (and t for the last)
                def emit_xproj(tt):
                    pm = ps_mm.tile([128, 512], F32, tag="pm")
                    for d in range(DC):
                        nc.tensor.matmul(pm[0:96, :], wx_sb[d][:],
                                         xb[d][:, 512 * tt:512 * (tt + 1)],
                                         start=(d == 0), stop=(d == DC - 1))
                    nc.scalar.activation(xdbl_p[:, 512 * tt:512 * (tt + 1)],
                                         pm[0:96, :], AF.Copy)
                if t > 0:
                    emit_xproj(t - 1)
                if t == TCH - 1:
                    emit_xproj(t)
        pX.release()

        # ====== Phase D: AllReduce (z-projection fills the latency) ======
        pDE = tc.alloc_tile_pool(name="pDE", bufs=1, side="right")
        bbc = pDE.tile([128, L], BF16)
        cbc = pDE.tile([128, L], BF16)
        dtlow = pDE.tile([DT_RANK, L], BF16)
        with tc.tile_pool(name="pD", bufs=2) as pD:
            xdbl = pD.tile([96, L], F32, tag="xdbl", bufs=1)
            bounce_i = dram.tile([96, L], F32)
            bounce_o = dram.tile([96, L], F32)
            nc.sync.dma_start(bounce_i[:], xdbl_p[:])
            nc.gpsimd.collective_compute(
                "AllReduce", OP.add,
                replica_groups=[[0, 1, 2, 3], [4, 5, 6, 7]],
                ins=[bounce_i.opt()], outs=[bounce_o.opt()])
            nc.sync.dma_start(xdbl[:], bounce_o[:])

            for t in range(TCH):     # z-half of in_proj, overlaps AllReduce
                for d in range(DC):
                    pm = ps_mm.tile([128, 512], F32, tag="pm")
                    for k in range(KC):
                        nc.tensor.matmul(pm[:],
                                         winz_sb[k][:, 128 * d:128 * (d + 1)],
                                         hT[k][:, 512 * t:512 * (t + 1)],
                                         start=(k == 0), stop=(k == KC - 1))
                    nc.scalar.activation(zg[d][:, 512 * t:512 * (t + 1)],
                                         pm[:], AF.Silu)

            nc.vector.tensor_copy(dtlow[:], xdbl[0:DT_RANK, :])
            bc_bf = pD.tile([32, L], BF16, tag="bcbf", bufs=1)
            nc.vector.tensor_copy(bc_bf[:], xdbl[DT_RANK:96, :])
            for i in range(NSUB):
                nc.sync.dma_start(bbc[NST * i:NST * (i + 1), :],
                                  bc_bf[0:NST, :])
                nc.sync.dma_start(cbc[NST * i:NST * (i + 1), :],
                                  bc_bf[NST:32, :])
        pW.release()
        ps_mm.release()

        # ====== Phase F: dt path + selective scan (fused per d) ======
        pY = tc.alloc_tile_pool(name="pY", bufs=1, side="right")
        yg = [pY.tile([128, L], BF16, tag=f"yg{d}", name=f"yg{d}")
              for d in range(DC)]
        with tc.tile_pool(name="pF", bufs=2) as pF, \
             tc.tile_pool(name="ps_y", bufs=1, space="PSUM") as ps_y, \
             tc.tile_pool(name="ps_dt", bufs=4, space="PSUM") as ps_dt:
            dt_ds = {}
            ub_ds = {}

            def emit_prep(d):
                # dt = softplus(dt_proj(dtlow) + bias) via exp + log1p series
                # (u = exp(v + bias) <= ~0.12, 4-term series exact to ~4e-6)
                u_t = pF.tile([128, L], F32, tag="u_t", bufs=1, name="u_t")
                for t in range(TCH):
                    pm = ps_dt.tile([128, 512], F32, tag="pmF", name="pm")
                    nc.tensor.matmul(pm[:], wdt_sb[:, 128 * d:128 * (d + 1)],
                                     dtlow[:, 512 * t:512 * (t + 1)],
                                     start=True, stop=True)
                    nc.scalar.activation(u_t[:, 512 * t:512 * (t + 1)],
                                         pm[:], AF.Exp,
                                         bias=dtb_sb[:, d:d + 1])
                t1 = pF.tile([128, L], F32, tag="t1", bufs=1, name="t1")
                nc.vector.tensor_scalar(t1[:], u_t[:], -0.25, 1.0 / 3.0,
                                        OP.mult, OP.add)
                nc.vector.tensor_mul(t1[:], t1[:], u_t[:])
                nc.vector.tensor_scalar(t1[:], t1[:], 1.0, -0.5,
                                        OP.mult, OP.add)
                nc.vector.tensor_mul(t1[:], t1[:], u_t[:])
                dt_d = pF.tile([128, L], F32R, tag="dt_d", bufs=2, name="dt_d")
                nc.vector.scalar_tensor_tensor(dt_d[:], t1[:], 1.0, u_t[:],
                                               OP.add, OP.mult)
                ub_d = pF.tile([128, L], BF16, tag="ub_d", bufs=2, name="ub_d")
                nc.vector.tensor_mul(ub_d[:], dt_d[:].bitcast(F32), xb[d][:])
                ub_sc = dram.tile([128, L], BF16, tag="ub_sc", bufs=2,
                                  name="ub_sc")
                nc.sync.dma_start(ub_sc[:], ub_d[:])
                dt_ds[d] = dt_d
                ub_ds[d] = ub_sc

            emit_prep(0)
            for d in range(DC):
                dt_d = dt_ds[d]
                ub_sd = ub_ds[d]
                ypsum = ps_y.tile([128, L], F32, tag="ypsum")
                for q in range(SPC // 4):
                    # replicate ub rows for 4 subtiles: DRAM->SBUF, the
                    # free-form DRAM source AP supplies (i, s, t) order
                    quad = pF.tile([128, 4 * L], BF16, tag="quad", bufs=2)
                    quad_v = quad[:].rearrange("(a b) (s t) -> a b s t",
                                               a=NSUB, s=4)
                    src_v = ub_sd[32 * q:32 * (q + 1), :].rearrange(
                        "(s i) t -> i s t", s=4)
                    for n in range(NST):
                        nc.sync.dma_start(quad_v[:, n, :, :], src_v)
                    for sq in range(4):
                        sidx = 4 * q + sq
                        a_t = pF.tile([128, L], F32, tag="a", bufs=2)
                        for t in range(TCH):
                            o = 512 * t
                            pm = ps_dt.tile([128, 512], F32, tag="pmF")
                            nc.tensor.matmul(
                                pm[:], exp_sb[:, 128 * sidx:128 * (sidx + 1)],
                                dt_d[:, o:o + 512], start=True, stop=True)
                            nc.scalar.activation(
                                a_t[:, o:o + 512], pm[:], AF.Exp,
                                scale=asc_sb[:,
                                             d * SPC + sidx:d * SPC + sidx + 1])
                        b_t = pF.tile([128, L], BF16, tag="b")
                        nc.vector.tensor_mul(b_t[:],
                                             quad[:, L * sq:L * (sq + 1)],
                                             bbc[:])
                        h_t = pF.tile([128, L], BF16, tag="h")
                        nc.vector.tensor_tensor_scan(h_t[:], a_t[:], b_t[:],
                                                     0.0, OP.mult, OP.add)
                        hc = pF.tile([128, L], BF16, tag="hc")
                        nc.vector.tensor_mul(hc[:], h_t[:], cbc[:])
                        for t in range(TCH):
                            nc.tensor.matmul(
                                ypsum[:, 512 * t:512 * (t + 1)],
                                sel_sb[:, 128 * sidx:128 * (sidx + 1)],
                                hc[:, 512 * t:512 * (t + 1)],
                                start=(sidx == 0), stop=(sidx == SPC - 1),
                                skip_group_check=True)
                    if q == 1 and d + 1 < DC:
                        emit_prep(d + 1)
                # y2 = D*x + y ;  yg = y2 * silu(z)
                for hh in range(2):
                    o = 1024 * hh
                    y2 = pF.tile([128, 1024], F32, tag="y2", bufs=1)
                    nc.vector.scalar_tensor_tensor(
                        y2[:], xb[d][:, o:o + 1024], dpar_sb[:, d:d + 1],
                        ypsum[:, o:o + 1024], OP.mult, OP.add)
                    nc.vector.tensor_mul(yg[d][:, o:o + 1024], y2[:],
                                         zg[d][:, o:o + 1024])
        # ====== Phase G: out_proj ======
        with tc.tile_pool(name="pG", bufs=3) as pG, \
             tc.tile_pool(name="ps_g", bufs=4, space="PSUM") as ps_g:
            for tb in range(RT):
                osb = pG.tile([128, D_MODEL], F32, tag="osb")
                for e in range(2):
                    pm = ps_g.tile([128, 512], F32, tag="pmG")
                    for d in range(DC):
                        nc.tensor.matmul(
                            pm[:], yg[d][:, 128 * tb:128 * (tb + 1)],
                            wout_sb[d][:, 512 * e:512 * (e + 1)],
                            start=(d == 0), stop=(d == DC - 1))
                    if e == 0:
                        nc.scalar.activation(osb[:, 512 * e:512 * (e + 1)],
                                             pm[:], AF.Copy)
                    else:
                        nc.vector.tensor_copy(osb[:, 512 * e:512 * (e + 1)],
                                              pm[:])
                nc.sync.dma_start(out_part[128 * tb:128 * (tb + 1), :],
                                  osb[:])
        pY.release()
        pDE.release()
        pBC.release()
        cst.release()
        dram.release()
    nc.compile()

    return nc


_NC_CACHE = None


def _get_nc():
    global _NC_CACHE
    if _NC_CACHE is None:
        _NC_CACHE = _build()
    return _NC_CACHE


def kernel(input_ids=None, hidden_states=None, residual=None, norm_w=None,
           in_proj_w=None, conv_w=None, conv_b=None, x_proj_w=None,
           dt_proj_w=None, dt_proj_b=None, A_log=None, D_param=None,
           out_proj_w=None, **kwargs):
    import ml_dtypes
    bf16 = np.dtype(ml_dtypes.bfloat16)

    hs = np.asarray(hidden_states, np.float32)
    rs = np.asarray(residual, np.float32)
    ipw = np.asarray(in_proj_w, np.float32)
    cw = np.asarray(conv_w, np.float32)
    cb = np.asarray(conv_b, np.float32)
    xpw = np.asarray(x_proj_w, np.float32)
    dpw = np.asarray(dt_proj_w, np.float32)
    dpb = np.asarray(dt_proj_b, np.float32)
    al = np.asarray(A_log, np.float32)
    dpr = np.asarray(D_param, np.float32)
    opw = np.asarray(out_proj_w, np.float32)
    nw = np.asarray(norm_w, np.float32)

    def colpack(v):  # [DLOC] -> [128, DC], col d = v[d*128:(d+1)*128]
        return np.ascontiguousarray(v.reshape(DC, 128).T).astype(np.float32)

    selm = np.zeros((128, SPC * 128), np.float32)
    expm = np.zeros((128, SPC * 128), np.float32)
    for s in range(SPC):
        for i in range(NSUB):
            m = s * NSUB + i
            for n in range(NST):
                p = i * NST + n
                selm[p, s * 128 + m] = 1.0
                expm[m, s * 128 + p] = 1.0
    ident = np.eye(128, dtype=np.float32)
    normw_t = np.ascontiguousarray(nw.reshape(KC, 128).T).astype(np.float32)

    nc = _get_nc()
    in_maps = []
    for c in range(N_CORES):
        b, k = c // TPG, c % TPG
        sl = slice(k * DLOC, (k + 1) * DLOC)
        slz = slice(D_INNER + k * DLOC, D_INNER + (k + 1) * DLOC)

        conv4 = cw[sl, 0, :]                       # [DLOC, 4]
        convw_t = np.ascontiguousarray(
            conv4.reshape(DC, 128, DCONV).transpose(1, 0, 2).reshape(
                128, DC * DCONV)).astype(np.float32)

        A = -np.exp(al[sl])                        # [DLOC, 16]
        a_sc = np.zeros((128, DC * SPC), np.float32)
        for d in range(DC):
            for s in range(SPC):
                rows = A[d * 128 + s * NSUB: d * 128 + (s + 1) * NSUB, :]
                a_sc[:, d * SPC + s] = rows.reshape(128)

        in_maps.append(dict(
            hid=np.ascontiguousarray(hs[b]),
            res=np.ascontiguousarray(rs[b]),
            winx=np.ascontiguousarray(ipw[sl].T * nw[:, None]).astype(bf16),
            winz=np.ascontiguousarray(ipw[slz].T * nw[:, None]).astype(bf16),
            wxT=np.ascontiguousarray(xpw[:, sl].T).astype(bf16),
            wdtT=np.ascontiguousarray(dpw[sl].T).astype(bf16),
            woutT=np.ascontiguousarray(opw[:, sl].T).astype(bf16),
            convw=convw_t,
            convb=colpack(cb[sl]),
            dtb=colpack(dpb[sl]),
            dpar=colpack(dpr[sl]),
            a_sc=a_sc,
            normw=normw_t,
            selm=selm.astype(bf16),
            expm=expm,
            ident=ident,
        ))

    res = run_bass_kernel_spmd(nc, in_maps, core_ids=list(range(N_CORES)))
    outs = [res.results[c]["out_part"] for c in range(N_CORES)]
    full = np.stack([
        sum(outs[b * TPG + k] for k in range(TPG)) for b in range(BATCH)
    ]).astype(np.float32)
    return full



# revision 21
# speedup vs baseline: 1.0782x; 1.0782x over previous
"""Mamba block kernel for Trainium2 (8 NeuronCores).

Sharding: batch (2-way) x tensor-parallel over d_inner (4-way).
Core c handles batch c//4 and d_inner channels [(c%4)*512, (c%4+1)*512).
Host folds norm_w into in_proj, pre-adds hidden+residual (bf16), and sums
the 4 TP partial outputs per batch.

Device pipeline per core (one NEFF, phases overlap via Tile scheduling):
  A. RMSNorm of r=hid+res (ACT square-accumulate variance), PE-transpose
     via bf16 identity matmuls -> hT_all [1024, L] bf16 (k-major columns)
  B. in_proj x-half (k-outer bf16 matmuls, 1024-wide moving), causal
     depthwise conv as 4 shifted diag-matmuls on PE accumulating in PSUM
     (SiLU fused in the ACT eviction), x_proj partials
  D. AllReduce of bf16 x_dbl partials (groups [[0-3],[4-7]], DRAM bounce);
     the z-half of in_proj + SiLU runs under the collective's latency
  F. per d-chunk (software-pipelined):
       dt = softplus via ACT: u = Exp(dt_proj + bias), dt = Ln(u + 1)
       ub = dt*x -> 4 DRAM copies -> per-quad expansion DMAs (4 per quad)
       selective scan over 16 expanded tiles [128=(8 d x 16 n), L]:
         a = exp(dt*A): PE f32r replicate-matmul, ACT Exp w/ A scale
         b = ub_exp*B broadcast (DVE bf16 2x)
         h = tensor_tensor_scan on DVE (1x, the hard floor)
         hc = h*C (split DVE / GpSimd Pool to balance engine load)
         y = sel 0/1-matmuls accumulating 16 subtiles into one PSUM tile
       y2 = D*x + y (DVE stt), yg = y2*silu(z) (Pool)
  G. out_proj partial (yg-block stationary, wout moving) -> [L, 1024] f32
"""

import sys

sys.path.insert(0, "/opt/trn_rl_repo")

import numpy as np

import concourse.bacc as bacc
import concourse.tile as tile
from concourse import mybir
from concourse.bass_utils import run_bass_kernel_spmd

F32 = mybir.dt.float32
F32R = mybir.dt.float32r
BF16 = mybir.dt.bfloat16
AF = mybir.ActivationFunctionType
OP = mybir.AluOpType

D_MODEL = 1024
D_INNER = 2048
NST = 16          # d_state
DT_RANK = 64
DCONV = 4
BATCH = 2
L = 2048
EPS = 1e-5

N_CORES = 8
TPG = 4                    # tensor-parallel group size
DLOC = D_INNER // TPG      # 512 channels per core
DC = DLOC // 128           # 4 partition chunks of x-channels
KC = D_MODEL // 128        # 8 contraction chunks
RT = L // 128              # 16 row tiles
NSUB = 128 // NST          # 8 d-channels per expanded tile
SPC = 128 // NSUB          # 16 subtiles per d-chunk

# hc-mul engine assignment: subtile (d*SPC+s) goes to Pool unless in this set
HC_DVE = frozenset()


def _build():
    nc = bacc.Bacc("TRN2", target_bir_lowering=False, debug=False,
                   enable_asserts=True, num_devices=N_CORES)

    def din(name, shape, dt=F32):
        return nc.dram_tensor(name, shape, dt, kind="ExternalInput").ap()

    rin = din("rin", [L, D_MODEL], BF16)        # hid+res, host-added
    winx = din("winx", [D_MODEL, DLOC], BF16)   # in_proj_w[x-slice].T * nw
    winz = din("winz", [D_MODEL, DLOC], BF16)   # in_proj_w[z-slice].T * nw
    wxT = din("wxT", [DLOC, 96], BF16)          # x_proj_w[:, slice].T
    wdtT = din("wdtT", [DT_RANK, DLOC], BF16)   # dt_proj_w[slice].T
    woutT = din("woutT", [DLOC, D_MODEL], BF16)  # out_proj_w[:, slice].T
    convd = din("convd", [128, DC * DCONV * 128], BF16)  # diag stationaries
    convb = din("convb", [128, DC])
    dtb = din("dtb", [128, DC])
    dpar = din("dpar", [128, DC])
    a_sc = din("a_sc", [128, DC * SPC])         # per-tile A scale column
    selm = din("selm", [128, SPC * 128], BF16)  # 16 selection matrices
    expm = din("expm", [128, SPC * 128], F32R)  # 16 expansion matrices
    identb = din("identb", [128, 128], BF16)

    out_part = nc.dram_tensor("out_part", [L, D_MODEL], F32,
                              kind="ExternalOutput").ap()

    with tile.TileContext(nc) as tc:
        cst = tc.alloc_tile_pool(name="cst", bufs=1)
        dram = tc.alloc_tile_pool(name="dram", bufs=1, space="DRAM")
        pW = tc.alloc_tile_pool(name="pW", bufs=1)

        # ---- constants / weights to SBUF ----
        convd_sb = cst.tile([128, DC * DCONV * 128], BF16)
        nc.sync.dma_start(convd_sb[:], convd[:])
        convb_sb = cst.tile([128, DC], F32)
        nc.sync.dma_start(convb_sb[:], convb[:])
        dtb_sb = cst.tile([128, DC], F32)
        nc.sync.dma_start(dtb_sb[:], dtb[:])
        dpar_sb = cst.tile([128, DC], F32)
        nc.sync.dma_start(dpar_sb[:], dpar[:])
        asc_sb = cst.tile([128, DC * SPC], F32)
        nc.sync.dma_start(asc_sb[:], a_sc[:])
        sel_sb = cst.tile([128, SPC * 128], BF16)
        nc.sync.dma_start(sel_sb[:], selm[:])
        exp_sb = cst.tile([128, SPC * 128], F32R)
        nc.sync.dma_start(exp_sb[:], expm[:])
        id_sb = cst.tile([128, 128], BF16)
        nc.sync.dma_start(id_sb[:], identb[:])
        eps_sb = cst.tile([128, 1], F32)
        nc.vector.memset(eps_sb[:], EPS)
        wx_sb = [cst.tile([128, 96], BF16, tag=f"wx{d}", name=f"wx{d}")
                 for d in range(DC)]
        for d in range(DC):
            nc.sync.dma_start(wx_sb[d][:], wxT[128 * d:128 * (d + 1), :])
        wdt_sb = cst.tile([DT_RANK, DLOC], BF16)
        nc.sync.dma_start(wdt_sb[:], wdtT[:])
        wout_sb = [cst.tile([128, D_MODEL], BF16, tag=f"wo{d}", name=f"wo{d}")
                   for d in range(DC)]
        for d in range(DC):
            nc.sync.dma_start(wout_sb[d][:], woutT[128 * d:128 * (d + 1), :])
        winx_sb = [pW.tile([128, DLOC], BF16, tag=f"winx{k}", name=f"winx{k}")
                   for k in range(KC)]
        winz_sb = [pW.tile([128, DLOC], BF16, tag=f"winz{k}", name=f"winz{k}")
                   for k in range(KC)]
        hT_all = pW.tile([128, KC * L], BF16)
        hT_v = hT_all[:].rearrange("p (k t) -> p k t", k=KC)

        # ====== Phase A: RMSNorm + transpose ======
        with tc.tile_pool(name="pA", bufs=2) as pA, \
             tc.tile_pool(name="pA2", bufs=2) as pA2, \
             tc.tile_pool(name="ps_tr", bufs=4, space="PSUM") as ps_tr:
            for rt in range(RT):
                t0 = 128 * rt
                ld = pA.tile([128, D_MODEL], BF16, tag="ld")
                nc.sync.dma_start(ld[:], rin[t0:t0 + 128, :])
                sq = pA2.tile([128, D_MODEL], BF16, tag="sq", bufs=1)
                st = pA2.tile([128, 1], F32, tag="st")
                nc.scalar.activation(sq[:], ld[:], AF.Square, accum_out=st[:])
                sg = pA2.tile([128, 1], F32, tag="sg")
                nc.scalar.activation(sg[:], st[:], AF.Sqrt,
                                     bias=eps_sb[:], scale=1.0 / D_MODEL)
                rstd = pA2.tile([128, 1], F32, tag="rstd")
                nc.vector.reciprocal(rstd[:], sg[:])
                hrow = pA2.tile([128, D_MODEL], BF16, tag="hrow")
                nc.vector.tensor_scalar_mul(hrow[:], ld[:], rstd[:])
                for c in range(2):
                    pt = ps_tr.tile([128, 512], BF16, tag="tr")
                    for j in range(4):
                        k = 4 * c + j
                        nc.tensor.transpose(pt[:, 128 * j:128 * (j + 1)],
                                            hrow[:, 128 * k:128 * (k + 1)],
                                            id_sb[:])
                    dst = hT_v[:, 4 * c:4 * (c + 1), t0:t0 + 128]
                    src = pt[:].rearrange("p (k t) -> p k t", k=4)
                    nc.scalar.activation(dst, src, AF.Copy)

        for k in range(KC):
            nc.sync.dma_start(winx_sb[k][:], winx[128 * k:128 * (k + 1), :])
            nc.sync.dma_start(winz_sb[k][:], winz[128 * k:128 * (k + 1), :])

        # ====== Phase B: in_proj x (k-outer) + conv on PE + x_proj ======
        pBC = tc.alloc_tile_pool(name="pBC", bufs=1, side="right")
        zg = [pBC.tile([128, L], BF16, tag=f"zg{d}", name=f"zg{d}")
              for d in range(DC)]
        xb = [pBC.tile([128, L], BF16, tag=f"xb{d}", name=f"xb{d}")
              for d in range(DC)]
        pDE = tc.alloc_tile_pool(name="pDE", bufs=1, side="right")
        xdbl_p = pDE.tile([96, L], BF16)
        xdbl_sb = pDE.tile([96, L], BF16)
        bbc = pDE.tile([128, L], BF16)
        cbc = pDE.tile([128, L], BF16)
        pX = tc.alloc_tile_pool(name="pX", bufs=1, side="right")
        xpad = [pX.tile([128, L + DCONV], BF16, tag=f"xpad{d}",
                        name=f"xpad{d}") for d in range(DC)]
        for d in range(DC):
            nc.vector.memset(xpad[d][:, 0:DCONV - 1], 0.0)

        ps_b = tc.alloc_tile_pool(name="ps_b", bufs=1, space="PSUM")

        def half_proj(w_sb, evict):
            # k-outer in_proj for one half; evict(d, t2, pm) consumes PSUM
            for dp in range(2):
                pms = [ps_b.tile([128, 1024], F32, tag=f"pmb{j}{t2}",
                                 name=f"pmb{j}{t2}")
                       for j in range(2) for t2 in range(2)]
                for k in range(KC):
                    for j in range(2):
                        d = 2 * dp + j
                        for t2 in range(2):
                            for h in range(2):
                                o = 1024 * t2 + 512 * h
                                nc.tensor.matmul(
                                    pms[2 * j + t2][:, 512 * h:512 * (h + 1)],
                                    w_sb[k][:, 128 * d:128 * (d + 1)],
                                    hT_v[:, k, o:o + 512],
                                    start=(k == 0), stop=(k == KC - 1))
                for j in range(2):
                    for t2 in range(2):
                        evict(2 * dp + j, t2, pms[2 * j + t2])

        def evict_x(d, t2, pm):
            o = DCONV - 1 + 1024 * t2
            nc.scalar.activation(xpad[d][:, o:o + 1024], pm[:], AF.Copy)

        half_proj(winx_sb, evict_x)

        # conv: 4 shifted diag matmuls per (d, t2), SiLU fused in eviction
        for d in range(DC):
            for t2 in range(2):
                pm = ps_b.tile([128, 1024], F32, tag="pmb00")
                for k in range(DCONV):
                    for h in range(2):
                        o = 1024 * t2 + 512 * h + k
                        nc.tensor.matmul(
                            pm[:, 512 * h:512 * (h + 1)],
                            convd_sb[:, 128 * (DCONV * d + k):
                                     128 * (DCONV * d + k + 1)],
                            xpad[d][:, o:o + 512],
                            start=(k == 0), stop=(k == DCONV - 1))
                nc.scalar.activation(xb[d][:, 1024 * t2:1024 * (t2 + 1)],
                                     pm[:], AF.Silu, bias=convb_sb[:, d:d + 1])
        pX.release()

        # x_proj partials
        for t2 in range(2):
            pm = ps_b.tile([128, 1024], F32, tag="pmb01")
            for d in range(DC):
                for h in range(2):
                    o = 1024 * t2 + 512 * h
                    nc.tensor.matmul(pm[0:96, 512 * h:512 * (h + 1)],
                                     wx_sb[d][:], xb[d][:, o:o + 512],
                                     start=(d == 0), stop=(d == DC - 1))
            nc.scalar.activation(xdbl_p[:, 1024 * t2:1024 * (t2 + 1)],
                                 pm[0:96, :], AF.Copy)

        # ====== Phase D: AllReduce (bf16); z-half fills the latency ======
        bounce_i = dram.tile([96, L], BF16)
        bounce_o = dram.tile([96, L], BF16)
        nc.sync.dma_start(bounce_i[:], xdbl_p[:])
        nc.gpsimd.collective_compute(
            "AllReduce", OP.add,
            replica_groups=[[0, 1, 2, 3], [4, 5, 6, 7]],
            ins=[bounce_i.opt()], outs=[bounce_o.opt()])
        nc.sync.dma_start(xdbl_sb[:], bounce_o[:])


        def evict_z(d, t2, pm):
            nc.scalar.activation(zg[d][:, 1024 * t2:1024 * (t2 + 1)],
                                 pm[:], AF.Silu)

        half_proj(winz_sb, evict_z)

        dtlow = xdbl_sb[0:64, :]
        for i in range(NSUB):
            nc.sync.dma_start(bbc[NST * i:NST * (i + 1), :],
                              xdbl_sb[64:80, :])
            nc.sync.dma_start(cbc[NST * i:NST * (i + 1), :],
                              xdbl_sb[80:96, :])
        pW.release()
        ps_b.release()

        # ====== Phase F: dt path + selective scan (fused per d) ======
        pY = tc.alloc_tile_pool(name="pY", bufs=1, side="right")
        yg = [pY.tile([128, L], BF16, tag=f"yg{d}", name=f"yg{d}")
              for d in range(DC)]
        with tc.tile_pool(name="pF", bufs=2) as pF, \
             tc.tile_pool(name="pQ", bufs=2) as pQ, \
             tc.tile_pool(name="ps_y", bufs=1, space="PSUM") as ps_y, \
             tc.tile_pool(name="ps_f", bufs=2, space="PSUM") as ps_f:
            dt_ds = {}
            ub_ds = {}

            def emit_prep(d):
                # dt = softplus(dt_proj(dtlow)+bias) = Ln(1 + Exp(.+bias))
                u_t = pF.tile([128, L], F32, tag="u_t", bufs=1, name="u_t")
                for t2 in range(2):
                    pm = ps_f.tile([128, 1024], F32, tag="pa", name="pa")
                    for h in range(2):
                        o = 1024 * t2 + 512 * h
                        nc.tensor.matmul(
                            pm[:, 512 * h:512 * (h + 1)],
                            wdt_sb[:, 128 * d:128 * (d + 1)],
                            dtlow[:, o:o + 512], start=True, stop=True,
                            skip_group_check=True)
                    nc.scalar.activation(u_t[:, 1024 * t2:1024 * (t2 + 1)],
                                         pm[:], AF.Exp,
                                         bias=dtb_sb[:, d:d + 1])
                dt_d = pF.tile([128, L], F32R, tag="dt_d", bufs=2,
                               name="dt_d")
                nc.scalar.activation(dt_d[:], u_t[:], AF.Ln, bias=1.0)
                ub_d = pF.tile([128, L], BF16, tag="ub_d", bufs=2, name="ub_d")
                nc.vector.tensor_mul(ub_d[:], dt_d[:].bitcast(F32), xb[d][:])
                ub_sc = dram.tile([128, L], BF16, tag="ub_sc", bufs=2,
                                  name="ub_sc")
                nc.sync.dma_start(ub_sc[:], ub_d[:])
                dt_ds[d] = dt_d
                ub_ds[d] = ub_sc

            emit_prep(0)
            for d in range(DC):
                dt_d = dt_ds[d]
                ub_sc = ub_ds[d]
                ypsum = ps_y.tile([128, L], F32, tag="ypsum")
                for q in range(SPC // 4):
                    quad = pQ.tile([128, 4 * L], BF16, tag="quad", bufs=2)
                    quad_v = quad[:].rearrange(
                        "(i n) (s t) -> i n s t", i=NSUB, s=4)
                    src_r = ub_sc[32 * q:32 * (q + 1), :].rearrange(
                        "(s i) t -> i s t", s=4)
                    for n in range(NST):
                        nc.sync.dma_start(quad_v[:, n, :, :], src_r)
                    for sq in range(4):
                        sidx = 4 * q + sq
                        a_t = pF.tile([128, L], F32, tag="a", bufs=2)
                        for t2 in range(2):
                            pm = ps_f.tile([128, 1024], F32, tag="pa")
                            for h in range(2):
                                o = 1024 * t2 + 512 * h
                                nc.tensor.matmul(
                                    pm[:, 512 * h:512 * (h + 1)],
                                    exp_sb[:, 128 * sidx:128 * (sidx + 1)],
                                    dt_d[:, o:o + 512],
                                    start=True, stop=True,
                                    skip_group_check=True)
                            nc.scalar.activation(
                                a_t[:, 1024 * t2:1024 * (t2 + 1)], pm[:],
                                AF.Exp,
                                scale=asc_sb[:,
                                             d * SPC + sidx:d * SPC + sidx + 1])
                        b_t = pF.tile([128, L], BF16, tag="b")
                        nc.vector.tensor_mul(b_t[:],
                                             quad[:, L * sq:L * (sq + 1)],
                                             bbc[:])
                        h_t = pF.tile([128, L], BF16, tag="h")
                        nc.vector.tensor_tensor_scan(h_t[:], a_t[:], b_t[:],
                                                     0.0, OP.mult, OP.add)
                        hc = pF.tile([128, L], BF16, tag="hc")
                        if d * SPC + sidx in HC_DVE:
                            nc.vector.tensor_mul(hc[:], h_t[:], cbc[:])
                        else:
                            nc.gpsimd.tensor_mul(hc[:], h_t[:], cbc[:])
                        for tq in range(4):
                            nc.tensor.matmul(
                                ypsum[:, 512 * tq:512 * (tq + 1)],
                                sel_sb[:, 128 * sidx:128 * (sidx + 1)],
                                hc[:, 512 * tq:512 * (tq + 1)],
                                start=(sidx == 0), stop=(sidx == SPC - 1),
                                skip_group_check=True)
                    if q == 1 and d + 1 < DC:
                        emit_prep(d + 1)
                # y2 = D*x + y ;  yg = y2 * silu(z)
                for hh in range(2):
                    o = 1024 * hh
                    y2 = pF.tile([128, 1024], F32, tag="y2", bufs=2)
                    nc.vector.scalar_tensor_tensor(
                        y2[:], xb[d][:, o:o + 1024], dpar_sb[:, d:d + 1],
                        ypsum[:, o:o + 1024], OP.mult, OP.add)
                    nc.gpsimd.tensor_mul(yg[d][:, o:o + 1024], y2[:],
                                         zg[d][:, o:o + 1024])
        # ====== Phase G: out_proj ======
        with tc.tile_pool(name="pG", bufs=3) as pG, \
             tc.tile_pool(name="ps_g", bufs=2, space="PSUM") as ps_g:
            for tb in range(RT):
                pm = ps_g.tile([128, D_MODEL], F32, tag="pmG")
                for d in range(DC):
                    for h in range(2):
                        nc.tensor.matmul(
                            pm[:, 512 * h:512 * (h + 1)],
                            yg[d][:, 128 * tb:128 * (tb + 1)],
                            wout_sb[d][:, 512 * h:512 * (h + 1)],
                            start=(d == 0), stop=(d == DC - 1))
                osb = pG.tile([128, D_MODEL], F32, tag="osb")
                nc.scalar.activation(osb[:], pm[:], AF.Copy)
                nc.sync.dma_start(out_part[128 * tb:128 * (tb + 1), :],
                                  osb[:])
        pY.release()
        pDE.release()
        pBC.release()
        cst.release()
        dram.release()
    nc.compile()

    return nc


_NC_CACHE = None


def _get_nc():
    global _NC_CACHE
    if _NC_CACHE is None:
        _NC_CACHE = _build()
    return _NC_CACHE


def kernel(input_ids=None, hidden_states=None, residual=None, norm_w=None,
           in_proj_w=None, conv_w=None, conv_b=None, x_proj_w=None,
           dt_proj_w=None, dt_proj_b=None, A_log=None, D_param=None,
           out_proj_w=None, **kwargs):
    import ml_dtypes
    bf16 = np.dtype(ml_dtypes.bfloat16)

    hs = np.asarray(hidden_states, np.float32)
    rs = np.asarray(residual, np.float32)
    ipw = np.asarray(in_proj_w, np.float32)
    cw = np.asarray(conv_w, np.float32)
    cb = np.asarray(conv_b, np.float32)
    xpw = np.asarray(x_proj_w, np.float32)
    dpw = np.asarray(dt_proj_w, np.float32)
    dpb = np.asarray(dt_proj_b, np.float32)
    al = np.asarray(A_log, np.float32)
    dpr = np.asarray(D_param, np.float32)
    opw = np.asarray(out_proj_w, np.float32)
    nw = np.asarray(norm_w, np.float32)

    r_full = hs + rs                               # host-side residual add

    def colpack(v):  # [DLOC] -> [128, DC], col d = v[d*128:(d+1)*128]
        return np.ascontiguousarray(v.reshape(DC, 128).T).astype(np.float32)

    selm = np.zeros((128, SPC * 128), np.float32)
    expm = np.zeros((128, SPC * 128), np.float32)
    for s in range(SPC):
        for i in range(NSUB):
            m = s * NSUB + i
            for n in range(NST):
                p = i * NST + n
                selm[p, s * 128 + m] = 1.0
                expm[m, s * 128 + p] = 1.0
    identb = np.eye(128, dtype=np.float32)

    nc = _get_nc()
    in_maps = []
    for c in range(N_CORES):
        b, k = c // TPG, c % TPG
        sl = slice(k * DLOC, (k + 1) * DLOC)
        slz = slice(D_INNER + k * DLOC, D_INNER + (k + 1) * DLOC)

        conv4 = cw[sl, 0, :]                       # [DLOC, 4]
        convd = np.zeros((128, DC * DCONV * 128), np.float32)
        for d in range(DC):
            for kk in range(DCONV):
                blk = DCONV * d + kk
                np.fill_diagonal(
                    convd[:, 128 * blk:128 * (blk + 1)],
                    conv4[128 * d:128 * (d + 1), kk])

        A = -np.exp(al[sl])                        # [DLOC, 16]
        a_sc = np.zeros((128, DC * SPC), np.float32)
        for d in range(DC):
            for s in range(SPC):
                rows = A[d * 128 + s * NSUB: d * 128 + (s + 1) * NSUB, :]
                a_sc[:, d * SPC + s] = rows.reshape(128)

        in_maps.append(dict(
            rin=r_full[b].astype(bf16),
            winx=np.ascontiguousarray(ipw[sl].T * nw[:, None]).astype(bf16),
            winz=np.ascontiguousarray(ipw[slz].T * nw[:, None]).astype(bf16),
            wxT=np.ascontiguousarray(xpw[:, sl].T).astype(bf16),
            wdtT=np.ascontiguousarray(dpw[sl].T).astype(bf16),
            woutT=np.ascontiguousarray(opw[:, sl].T).astype(bf16),
            convd=convd.astype(bf16),
            convb=colpack(cb[sl]),
            dtb=colpack(dpb[sl]),
            dpar=colpack(dpr[sl]),
            a_sc=a_sc,
            selm=selm.astype(bf16),
            expm=expm,
            identb=identb.astype(bf16),
        ))

    res = run_bass_kernel_spmd(nc, in_maps, core_ids=list(range(N_CORES)))
    outs = [res.results[c]["out_part"] for c in range(N_CORES)]
    full = np.stack([
        sum(outs[b * TPG + k] for k in range(TPG)) for b in range(BATCH)
    ]).astype(np.float32)
    return full


# revision 25
# speedup vs baseline: 1.2412x; 1.1512x over previous
"""Mamba block kernel for Trainium2 (8 NeuronCores).

Sharding: batch (2-way) x tensor-parallel over d_inner (4-way).
Core c handles batch c//4 and d_inner channels [(c%4)*512, (c%4+1)*512).
Host folds norm_w into in_proj, pre-adds hidden+residual (bf16), and sums
the 4 TP partial outputs per batch.

Device pipeline per core (one NEFF, phases overlap via Tile scheduling):
  A. RMSNorm of r=hid+res (ACT square-accumulate variance), PE-transpose
     via bf16 identity matmuls -> hT_all [1024, L] bf16 (k-major columns)
  B. in_proj x-half (k-outer bf16 matmuls, 1024-wide moving), causal
     depthwise conv as 4 shifted diag-matmuls on PE accumulating in PSUM
     (SiLU fused in the ACT eviction), x_proj partials
  D. AllReduce of bf16 x_dbl partials (groups [[0-3],[4-7]], DRAM bounce);
     the z-half of in_proj + SiLU runs under the collective's latency
  F. per d-chunk (software-pipelined):
       dt = softplus via ACT: u = Exp(dt_proj + bias), dt = Ln(u + 1)
       ub = dt*x -> 4 DRAM copies -> per-quad expansion DMAs (4 per quad)
       selective scan over 16 expanded tiles [128=(8 d x 16 n), L]:
         a = exp(dt*A): PE f32r replicate-matmul, ACT Exp w/ A scale
         b = ub_exp*B broadcast (DVE bf16 2x)
         h = tensor_tensor_scan on DVE (1x, the hard floor)
         hc = h*C (split DVE / GpSimd Pool to balance engine load)
         y = sel 0/1-matmuls accumulating 16 subtiles into one PSUM tile
       y2 = D*x + y (DVE stt), yg = y2*silu(z) (Pool)
  G. out_proj partial (yg-block stationary, wout moving) -> [L, 1024] f32
"""

import sys

sys.path.insert(0, "/opt/trn_rl_repo")

import numpy as np

import concourse.bacc as bacc
import concourse.tile as tile
from concourse import mybir
from concourse.bass_utils import run_bass_kernel_spmd

F32 = mybir.dt.float32
F32R = mybir.dt.float32r
BF16 = mybir.dt.bfloat16
AF = mybir.ActivationFunctionType
OP = mybir.AluOpType

D_MODEL = 1024
D_INNER = 2048
NST = 16          # d_state
DT_RANK = 64
DCONV = 4
BATCH = 2
L = 2048
EPS = 1e-5

N_CORES = 8
TPG = 4                    # tensor-parallel group size
DLOC = D_INNER // TPG      # 512 channels per core
DC = DLOC // 128           # 4 partition chunks of x-channels
KC = D_MODEL // 128        # 8 contraction chunks
RT = L // 128              # 16 row tiles
NSUB = 128 // NST          # 8 d-channels per expanded tile
SPC = 128 // NSUB          # 16 subtiles per d-chunk

# hc-mul engine assignment: subtile (d*SPC+s) goes to Pool unless in this set
HC_DVE = frozenset()


def _build():
    nc = bacc.Bacc("TRN2", target_bir_lowering=False, debug=False,
                   enable_asserts=True, num_devices=N_CORES)

    def din(name, shape, dt=F32):
        return nc.dram_tensor(name, shape, dt, kind="ExternalInput").ap()

    rin = din("rin", [L, D_MODEL], BF16)        # hid+res, host-added
    winx = din("winx", [D_MODEL, DLOC], BF16)   # in_proj_w[x-slice].T * nw
    winz = din("winz", [D_MODEL, DLOC], BF16)   # in_proj_w[z-slice].T * nw
    wxT = din("wxT", [DLOC, 96], BF16)          # x_proj_w[:, slice].T
    wdtT = din("wdtT", [DT_RANK, DLOC], BF16)   # dt_proj_w[slice].T
    woutT = din("woutT", [DLOC, D_MODEL], BF16)  # out_proj_w[:, slice].T
    convd = din("convd", [128, DC * DCONV * 128], BF16)  # diag stationaries
    convb = din("convb", [128, DC])
    dtb = din("dtb", [128, DC])
    dpar = din("dpar", [128, DC])
    a_sc = din("a_sc", [128, DC * SPC])         # per-tile A scale column
    selm = din("selm", [128, SPC * 128], BF16)  # 16 selection matrices
    expm = din("expm", [128, SPC * 128], F32R)  # 16 expansion matrices
    identb = din("identb", [128, 128], BF16)

    out_part = nc.dram_tensor("out_part", [L, D_MODEL], F32,
                              kind="ExternalOutput").ap()

    with tile.TileContext(nc) as tc:
        cst = tc.alloc_tile_pool(name="cst", bufs=1)
        dram = tc.alloc_tile_pool(name="dram", bufs=1, space="DRAM")
        pW = tc.alloc_tile_pool(name="pW", bufs=1)

        # ---- constants / weights to SBUF ----
        convd_sb = cst.tile([128, DC * DCONV * 128], BF16)
        nc.sync.dma_start(convd_sb[:], convd[:])
        convb_sb = cst.tile([128, DC], F32)
        nc.sync.dma_start(convb_sb[:], convb[:])
        dtb_sb = cst.tile([128, DC], F32)
        nc.sync.dma_start(dtb_sb[:], dtb[:])
        dpar_sb = cst.tile([128, DC], F32)
        nc.sync.dma_start(dpar_sb[:], dpar[:])
        asc_sb = cst.tile([128, DC * SPC], F32)
        nc.sync.dma_start(asc_sb[:], a_sc[:])
        sel_sb = cst.tile([128, SPC * 128], BF16)
        nc.sync.dma_start(sel_sb[:], selm[:])
        exp_sb = cst.tile([128, SPC * 128], F32R)
        nc.sync.dma_start(exp_sb[:], expm[:])
        id_sb = cst.tile([128, 128], BF16)
        nc.sync.dma_start(id_sb[:], identb[:])
        eps_sb = cst.tile([128, 1], F32)
        nc.vector.memset(eps_sb[:], EPS)
        wx_sb = [cst.tile([128, 96], BF16, tag=f"wx{d}", name=f"wx{d}")
                 for d in range(DC)]
        for d in range(DC):
            nc.sync.dma_start(wx_sb[d][:], wxT[128 * d:128 * (d + 1), :])
        wdt_sb = cst.tile([DT_RANK, DLOC], BF16)
        nc.sync.dma_start(wdt_sb[:], wdtT[:])
        wout_sb = [cst.tile([128, D_MODEL], BF16, tag=f"wo{d}", name=f"wo{d}")
                   for d in range(DC)]
        for d in range(DC):
            nc.sync.dma_start(wout_sb[d][:], woutT[128 * d:128 * (d + 1), :])
        winx_sb = [pW.tile([128, DLOC], BF16, tag=f"winx{k}", name=f"winx{k}")
                   for k in range(KC)]
        winz_sb = [pW.tile([128, DLOC], BF16, tag=f"winz{k}", name=f"winz{k}")
                   for k in range(KC)]
        hT_all = pW.tile([128, KC * L], BF16)
        hT_v = hT_all[:].rearrange("p (k t) -> p k t", k=KC)

        # ====== Phase A: RMSNorm + transpose ======
        with tc.tile_pool(name="pA", bufs=2) as pA, \
             tc.tile_pool(name="pA2", bufs=2) as pA2, \
             tc.tile_pool(name="ps_tr", bufs=4, space="PSUM") as ps_tr:
            for rt in range(RT):
                t0 = 128 * rt
                ld = pA.tile([128, D_MODEL], BF16, tag="ld")
                nc.sync.dma_start(ld[:], rin[t0:t0 + 128, :])
                sq = pA2.tile([128, D_MODEL], BF16, tag="sq", bufs=1)
                st = pA2.tile([128, 1], F32, tag="st")
                nc.scalar.activation(sq[:], ld[:], AF.Square, accum_out=st[:])
                sg = pA2.tile([128, 1], F32, tag="sg")
                nc.scalar.activation(sg[:], st[:], AF.Sqrt,
                                     bias=eps_sb[:], scale=1.0 / D_MODEL)
                rstd = pA2.tile([128, 1], F32, tag="rstd")
                nc.vector.reciprocal(rstd[:], sg[:])
                hrow = pA2.tile([128, D_MODEL], BF16, tag="hrow")
                nc.vector.tensor_scalar_mul(hrow[:], ld[:], rstd[:])
                for c in range(2):
                    pt = ps_tr.tile([128, 512], BF16, tag="tr")
                    for j in range(4):
                        k = 4 * c + j
                        nc.tensor.transpose(pt[:, 128 * j:128 * (j + 1)],
                                            hrow[:, 128 * k:128 * (k + 1)],
                                            id_sb[:])
                    dst = hT_v[:, 4 * c:4 * (c + 1), t0:t0 + 128]
                    src = pt[:].rearrange("p (k t) -> p k t", k=4)
                    nc.vector.tensor_copy(dst, src)

        for k in range(KC):
            nc.sync.dma_start(winx_sb[k][:], winx[128 * k:128 * (k + 1), :])
            nc.sync.dma_start(winz_sb[k][:], winz[128 * k:128 * (k + 1), :])

        # ====== Phase B: in_proj x (k-outer) + conv on PE + x_proj ======
        pBC = tc.alloc_tile_pool(name="pBC", bufs=1, side="right")
        zg = [pBC.tile([128, L], BF16, tag=f"zg{d}", name=f"zg{d}")
              for d in range(DC)]
        xb = [pBC.tile([128, L], BF16, tag=f"xb{d}", name=f"xb{d}")
              for d in range(DC)]
        pDE = tc.alloc_tile_pool(name="pDE", bufs=1, side="right")
        xdbl_p = pDE.tile([96, L], BF16)
        xdbl_sb = pDE.tile([96, L], BF16)
        bbc = pDE.tile([128, L], BF16)
        cbc = pDE.tile([128, L], BF16)
        pX = tc.alloc_tile_pool(name="pX", bufs=1, side="right")
        xpad = [pX.tile([128, L + DCONV], BF16, tag=f"xpad{d}",
                        name=f"xpad{d}") for d in range(DC)]
        for d in range(DC):
            nc.vector.memset(xpad[d][:, 0:DCONV - 1], 0.0)

        ps_b = tc.alloc_tile_pool(name="ps_b", bufs=1, space="PSUM")

        def emit_conv(d):
            # 4 shifted diag matmuls per (d, t2), SiLU fused in eviction
            for t2 in range(2):
                pm = ps_b.tile([128, 1024], F32, tag="pmb00", name="pm")
                for k in range(DCONV):
                    for h in range(2):
                        o = 1024 * t2 + 512 * h + k
                        nc.tensor.matmul(
                            pm[:, 512 * h:512 * (h + 1)],
                            convd_sb[:, 128 * (DCONV * d + k):
                                     128 * (DCONV * d + k + 1)],
                            xpad[d][:, o:o + 512],
                            start=(k == 0), stop=(k == DCONV - 1))
                nc.scalar.activation(xb[d][:, 1024 * t2:1024 * (t2 + 1)],
                                     pm[:], AF.Silu, bias=convb_sb[:, d:d + 1])

        def half_proj(w_sb, evict, post_dp=None):
            # k-outer in_proj for one half; evict(d, t2, pm) consumes PSUM
            for dp in range(2):
                pms = [ps_b.tile([128, 1024], F32, tag=f"pmb{j}{t2}",
                                 name=f"pmb{j}{t2}")
                       for j in range(2) for t2 in range(2)]
                for k in range(KC):
                    for j in range(2):
                        d = 2 * dp + j
                        for t2 in range(2):
                            for h in range(2):
                                o = 1024 * t2 + 512 * h
                                nc.tensor.matmul(
                                    pms[2 * j + t2][:, 512 * h:512 * (h + 1)],
                                    w_sb[k][:, 128 * d:128 * (d + 1)],
                                    hT_v[:, k, o:o + 512],
                                    start=(k == 0), stop=(k == KC - 1))
                for j in range(2):
                    for t2 in range(2):
                        evict(2 * dp + j, t2, pms[2 * j + t2])
                if post_dp is not None:
                    post_dp(dp)

        def evict_x(d, t2, pm):
            o = DCONV - 1 + 1024 * t2
            nc.scalar.activation(xpad[d][:, o:o + 1024], pm[:], AF.Copy)

        def post_x(dp):
            emit_conv(2 * dp)
            emit_conv(2 * dp + 1)

        half_proj(winx_sb, evict_x, post_dp=post_x)
        pX.release()

        # x_proj partials
        for t2 in range(2):
            pm = ps_b.tile([128, 1024], F32, tag="pmb01")
            for d in range(DC):
                for h in range(2):
                    o = 1024 * t2 + 512 * h
                    nc.tensor.matmul(pm[0:96, 512 * h:512 * (h + 1)],
                                     wx_sb[d][:], xb[d][:, o:o + 512],
                                     start=(d == 0), stop=(d == DC - 1))
            nc.scalar.activation(xdbl_p[:, 1024 * t2:1024 * (t2 + 1)],
                                 pm[0:96, :], AF.Copy)

        # ====== Phase D: AllReduce (bf16); z-half fills the latency ======
        bounce_i = dram.tile([96, L], BF16)
        bounce_o = dram.tile([96, L], BF16)
        nc.sync.dma_start(bounce_i[:], xdbl_p[:])
        nc.gpsimd.collective_compute(
            "AllReduce", OP.add,
            replica_groups=[[0, 1, 2, 3], [4, 5, 6, 7]],
            ins=[bounce_i.opt()], outs=[bounce_o.opt()])
        nc.sync.dma_start(xdbl_sb[:], bounce_o[:])


        def evict_z(d, t2, pm):
            nc.scalar.activation(zg[d][:, 1024 * t2:1024 * (t2 + 1)],
                                 pm[:], AF.Silu)

        half_proj(winz_sb, evict_z)

        dtlow = xdbl_sb[0:64, :]
        for i in range(NSUB):
            nc.sync.dma_start(bbc[NST * i:NST * (i + 1), :],
                              xdbl_sb[64:80, :])
            nc.sync.dma_start(cbc[NST * i:NST * (i + 1), :],
                              xdbl_sb[80:96, :])
        pW.release()
        ps_b.release()

        # ====== Phase F: dt path + selective scan (fused per d) ======
        pY = tc.alloc_tile_pool(name="pY", bufs=1, side="right")
        yg = [pY.tile([128, L], BF16, tag=f"yg{d}", name=f"yg{d}")
              for d in range(DC)]
        with tc.tile_pool(name="pF", bufs=2) as pF, \
             tc.tile_pool(name="pQ", bufs=2) as pQ, \
             tc.tile_pool(name="ps_y", bufs=1, space="PSUM") as ps_y, \
             tc.tile_pool(name="ps_f", bufs=2, space="PSUM") as ps_f:
            dt_ds = {}
            ub_ds = {}

            def emit_prep(d):
                # dt = softplus(dt_proj(dtlow)+bias) = Ln(1 + Exp(.+bias))
                u_t = pF.tile([128, L], F32, tag="u_t", bufs=1, name="u_t")
                for t2 in range(2):
                    pm = ps_f.tile([128, 1024], F32, tag="pa", name="pa")
                    for h in range(2):
                        o = 1024 * t2 + 512 * h
                        nc.tensor.matmul(
                            pm[:, 512 * h:512 * (h + 1)],
                            wdt_sb[:, 128 * d:128 * (d + 1)],
                            dtlow[:, o:o + 512], start=True, stop=True,
                            skip_group_check=True)
                    nc.scalar.activation(u_t[:, 1024 * t2:1024 * (t2 + 1)],
                                         pm[:], AF.Exp,
                                         bias=dtb_sb[:, d:d + 1])
                dt_d = pF.tile([128, L], F32R, tag="dt_d", bufs=2,
                               name="dt_d")
                nc.scalar.activation(dt_d[:], u_t[:], AF.Ln, bias=1.0)
                ub_d = pF.tile([128, L], BF16, tag="ub_d", bufs=2, name="ub_d")
                nc.vector.tensor_mul(ub_d[:], dt_d[:].bitcast(F32), xb[d][:])
                ub_sc = dram.tile([128, L], BF16, tag="ub_sc", bufs=2,
                                  name="ub_sc")
                nc.sync.dma_start(ub_sc[:], ub_d[:])
                dt_ds[d] = dt_d
                ub_ds[d] = ub_sc

            emit_prep(0)

            NG = DC * SPC  # 64 global subtiles
            quads = {}
            a_ts = {}
            h_ts = {}
            hc_ts = {}
            ypsums = {}

            def emit_quad_reads(qg):
                # quad qg covers subtiles 4qg..4qg+3 of d = qg//4
                quad = pQ.tile([128, 4 * L], BF16, tag="quad", bufs=2)
                quad_v = quad[:].rearrange(
                    "(i n) (s t) -> i n s t", i=NSUB, s=4)
                src_r = ub_ds[qg // 4][32 * (qg % 4):32 * (qg % 4 + 1),
                                       :].rearrange("(s i) t -> i s t", s=4)
                for n in range(NST):
                    nc.sync.dma_start(quad_v[:, n, :, :], src_r)
                quads[qg] = quad

            def emit_S0(g):
                # a = exp(dt*A): PE replicate-matmul + ACT Exp
                d, sidx = divmod(g, SPC)
                dt_d = dt_ds[d]
                a_t = pF.tile([128, L], F32, tag="a", bufs=2)
                for t2 in range(2):
                    pm = ps_f.tile([128, 1024], F32, tag="pa")
                    for h in range(2):
                        o = 1024 * t2 + 512 * h
                        nc.tensor.matmul(
                            pm[:, 512 * h:512 * (h + 1)],
                            exp_sb[:, 128 * sidx:128 * (sidx + 1)],
                            dt_d[:, o:o + 512], start=True, stop=True,
                            skip_group_check=True)
                    nc.scalar.activation(
                        a_t[:, 1024 * t2:1024 * (t2 + 1)], pm[:], AF.Exp,
                        scale=asc_sb[:, d * SPC + sidx:d * SPC + sidx + 1])
                a_ts[g] = a_t

            def emit_S12(g):
                # b = ub_exp*B (DVE), h = scan (DVE), hc = h*C (Pool/DVE)
                sq = g % 4
                quad = quads[g // 4]
                b_t = pF.tile([128, L], BF16, tag="b")
                nc.vector.tensor_mul(b_t[:], quad[:, L * sq:L * (sq + 1)],
                                     bbc[:])
                h_t = pF.tile([128, L], BF16, tag="h")
                nc.vector.tensor_tensor_scan(h_t[:], a_ts.pop(g)[:], b_t[:],
                                             0.0, OP.mult, OP.add)
                hc = pF.tile([128, L], BF16, tag="hc")
                if g % 9 == 4:
                    nc.vector.tensor_mul(hc[:], h_t[:], cbc[:])
                else:
                    nc.gpsimd.tensor_mul(hc[:], h_t[:], cbc[:])
                hc_ts[g] = hc

            def emit_S4(g):
                d, sidx = divmod(g, SPC)
                if sidx == 0:
                    ypsums[d] = ps_y.tile([128, L], F32, tag="ypsum",
                                          name="ypsum")
                hc = hc_ts.pop(g)
                for tq in range(4):
                    nc.tensor.matmul(
                        ypsums[d][:, 512 * tq:512 * (tq + 1)],
                        sel_sb[:, 128 * sidx:128 * (sidx + 1)],
                        hc[:, 512 * tq:512 * (tq + 1)],
                        start=(sidx == 0), stop=(sidx == SPC - 1),
                        skip_group_check=True)
                if sidx == SPC - 1:
                    # y2 = D*x + y ;  yg = y2 * silu(z)
                    for hh in range(2):
                        o = 1024 * hh
                        y2 = pF.tile([128, 1024], F32, tag="y2", bufs=2)
                        nc.vector.scalar_tensor_tensor(
                            y2[:], xb[d][:, o:o + 1024], dpar_sb[:, d:d + 1],
                            ypsums[d][:, o:o + 1024], OP.mult, OP.add)
                        nc.gpsimd.tensor_mul(yg[d][:, o:o + 1024], y2[:],
                                             zg[d][:, o:o + 1024])

            emit_quad_reads(0)
            for step in range(NG + 2):
                g0, g1, g2 = step, step - 1, step - 2
                if g0 < NG:
                    if g0 % SPC == 8 and g0 // SPC + 1 < DC:
                        emit_prep(g0 // SPC + 1)
                    gq = g0 + 2
                    if gq % 4 == 0 and gq < NG:
                        emit_quad_reads(gq // 4)
                    emit_S0(g0)
                if 0 <= g1 < NG:
                    emit_S12(g1)
                if 0 <= g2 < NG:
                    emit_S4(g2)
        # ====== Phase G: out_proj ======
        with tc.tile_pool(name="pG", bufs=3) as pG, \
             tc.tile_pool(name="ps_g", bufs=2, space="PSUM") as ps_g:
            for tb in range(RT):
                pm = ps_g.tile([128, D_MODEL], F32, tag="pmG")
                for d in range(DC):
                    for h in range(2):
                        nc.tensor.matmul(
                            pm[:, 512 * h:512 * (h + 1)],
                            yg[d][:, 128 * tb:128 * (tb + 1)],
                            wout_sb[d][:, 512 * h:512 * (h + 1)],
                            start=(d == 0), stop=(d == DC - 1))
                osb = pG.tile([128, D_MODEL], F32, tag="osb")
                nc.scalar.activation(osb[:], pm[:], AF.Copy)
                nc.sync.dma_start(out_part[128 * tb:128 * (tb + 1), :],
                                  osb[:])
        pY.release()
        pDE.release()
        pBC.release()
        cst.release()
        dram.release()
    nc.compile()

    return nc


_NC_CACHE = None


def _get_nc():
    global _NC_CACHE
    if _NC_CACHE is None:
        _NC_CACHE = _build()
    return _NC_CACHE


def kernel(input_ids=None, hidden_states=None, residual=None, norm_w=None,
           in_proj_w=None, conv_w=None, conv_b=None, x_proj_w=None,
           dt_proj_w=None, dt_proj_b=None, A_log=None, D_param=None,
           out_proj_w=None, **kwargs):
    import ml_dtypes
    bf16 = np.dtype(ml_dtypes.bfloat16)

    hs = np.asarray(hidden_states, np.float32)
    rs = np.asarray(residual, np.float32)
    ipw = np.asarray(in_proj_w, np.float32)
    cw = np.asarray(conv_w, np.float32)
    cb = np.asarray(conv_b, np.float32)
    xpw = np.asarray(x_proj_w, np.float32)
    dpw = np.asarray(dt_proj_w, np.float32)
    dpb = np.asarray(dt_proj_b, np.float32)
    al = np.asarray(A_log, np.float32)
    dpr = np.asarray(D_param, np.float32)
    opw = np.asarray(out_proj_w, np.float32)
    nw = np.asarray(norm_w, np.float32)

    r_full = hs + rs                               # host-side residual add

    def colpack(v):  # [DLOC] -> [128, DC], col d = v[d*128:(d+1)*128]
        return np.ascontiguousarray(v.reshape(DC, 128).T).astype(np.float32)

    selm = np.zeros((128, SPC * 128), np.float32)
    expm = np.zeros((128, SPC * 128), np.float32)
    for s in range(SPC):
        for i in range(NSUB):
            m = s * NSUB + i
            for n in range(NST):
                p = i * NST + n
                selm[p, s * 128 + m] = 1.0
                expm[m, s * 128 + p] = 1.0
    identb = np.eye(128, dtype=np.float32)

    nc = _get_nc()
    in_maps = []
    for c in range(N_CORES):
        b, k = c // TPG, c % TPG
        sl = slice(k * DLOC, (k + 1) * DLOC)
        slz = slice(D_INNER + k * DLOC, D_INNER + (k + 1) * DLOC)

        conv4 = cw[sl, 0, :]                       # [DLOC, 4]
        convd = np.zeros((128, DC * DCONV * 128), np.float32)
        for d in range(DC):
            for kk in range(DCONV):
                blk = DCONV * d + kk
                np.fill_diagonal(
                    convd[:, 128 * blk:128 * (blk + 1)],
                    conv4[128 * d:128 * (d + 1), kk])

        A = -np.exp(al[sl])                        # [DLOC, 16]
        a_sc = np.zeros((128, DC * SPC), np.float32)
        for d in range(DC):
            for s in range(SPC):
                rows = A[d * 128 + s * NSUB: d * 128 + (s + 1) * NSUB, :]
                a_sc[:, d * SPC + s] = rows.reshape(128)

        in_maps.append(dict(
            rin=r_full[b].astype(bf16),
            winx=np.ascontiguousarray(ipw[sl].T * nw[:, None]).astype(bf16),
            winz=np.ascontiguousarray(ipw[slz].T * nw[:, None]).astype(bf16),
            wxT=np.ascontiguousarray(xpw[:, sl].T).astype(bf16),
            wdtT=np.ascontiguousarray(dpw[sl].T).astype(bf16),
            woutT=np.ascontiguousarray(opw[:, sl].T).astype(bf16),
            convd=convd.astype(bf16),
            convb=colpack(cb[sl]),
            dtb=colpack(dpb[sl]),
            dpar=colpack(dpr[sl]),
            a_sc=a_sc,
            selm=selm.astype(bf16),
            expm=expm,
            identb=identb.astype(bf16),
        ))

    res = run_bass_kernel_spmd(nc, in_maps, core_ids=list(range(N_CORES)))
    outs = [res.results[c]["out_part"] for c in range(N_CORES)]
    full = np.stack([
        sum(outs[b * TPG + k] for k in range(TPG)) for b in range(BATCH)
    ]).astype(np.float32)
    return full


# revision 27
# speedup vs baseline: 1.2453x; 1.0033x over previous
"""Mamba block kernel for Trainium2 (8 NeuronCores).

Sharding: batch (2-way) x tensor-parallel over d_inner (4-way).
Core c handles batch c//4 and d_inner channels [(c%4)*512, (c%4+1)*512).
Host folds norm_w into in_proj, pre-adds hidden+residual (bf16), and sums
the 4 TP partial outputs per batch.

Device pipeline per core (one NEFF, phases overlap via Tile scheduling):
  A. RMSNorm of r=hid+res (ACT square-accumulate variance), PE-transpose
     via bf16 identity matmuls -> hT_all [1024, L] bf16 (k-major columns)
  B. in_proj x-half (k-outer bf16 matmuls, 1024-wide moving), causal
     depthwise conv as 4 shifted diag-matmuls on PE accumulating in PSUM
     (SiLU fused in the ACT eviction), x_proj partials
  D. AllReduce of bf16 x_dbl partials (groups [[0-3],[4-7]], DRAM bounce);
     the z-half of in_proj + SiLU runs under the collective's latency
  F. per d-chunk (software-pipelined):
       dt = softplus via ACT: u = Exp(dt_proj + bias), dt = Ln(u + 1)
       ub = dt*x -> 4 DRAM copies -> per-quad expansion DMAs (4 per quad)
       selective scan over 16 expanded tiles [128=(8 d x 16 n), L]:
         a = exp(dt*A): PE f32r replicate-matmul, ACT Exp w/ A scale
         b = ub_exp*B broadcast (DVE bf16 2x)
         h = tensor_tensor_scan on DVE (1x, the hard floor)
         hc = h*C (split DVE / GpSimd Pool to balance engine load)
         y = sel 0/1-matmuls accumulating 16 subtiles into one PSUM tile
       y2 = D*x + y (DVE stt), yg = y2*silu(z) (Pool)
  G. out_proj partial (yg-block stationary, wout moving) -> [L, 1024] f32
"""

import sys

sys.path.insert(0, "/opt/trn_rl_repo")

import numpy as np

import concourse.bacc as bacc
import concourse.tile as tile
from concourse import mybir
from concourse.bass_utils import run_bass_kernel_spmd

F32 = mybir.dt.float32
F32R = mybir.dt.float32r
BF16 = mybir.dt.bfloat16
AF = mybir.ActivationFunctionType
OP = mybir.AluOpType

D_MODEL = 1024
D_INNER = 2048
NST = 16          # d_state
DT_RANK = 64
DCONV = 4
BATCH = 2
L = 2048
EPS = 1e-5

N_CORES = 8
TPG = 4                    # tensor-parallel group size
DLOC = D_INNER // TPG      # 512 channels per core
DC = DLOC // 128           # 4 partition chunks of x-channels
KC = D_MODEL // 128        # 8 contraction chunks
RT = L // 128              # 16 row tiles
NSUB = 128 // NST          # 8 d-channels per expanded tile
SPC = 128 // NSUB          # 16 subtiles per d-chunk

# hc-mul engine assignment: subtile (d*SPC+s) goes to Pool unless in this set
HC_DVE = frozenset()


def _build():
    nc = bacc.Bacc("TRN2", target_bir_lowering=False, debug=False,
                   enable_asserts=True, num_devices=N_CORES)

    def din(name, shape, dt=F32):
        return nc.dram_tensor(name, shape, dt, kind="ExternalInput").ap()

    rin = din("rin", [L, D_MODEL], BF16)        # hid+res, host-added
    winx = din("winx", [D_MODEL, DLOC], BF16)   # in_proj_w[x-slice].T * nw
    winz = din("winz", [D_MODEL, DLOC], BF16)   # in_proj_w[z-slice].T * nw
    wxT = din("wxT", [DLOC, 96], BF16)          # x_proj_w[:, slice].T
    wdtT = din("wdtT", [DT_RANK, DLOC], BF16)   # dt_proj_w[slice].T
    woutT = din("woutT", [DLOC, D_MODEL], BF16)  # out_proj_w[:, slice].T
    convd = din("convd", [128, DC * DCONV * 128], BF16)  # diag stationaries
    convb = din("convb", [128, DC])
    dtb = din("dtb", [128, DC])
    dpar = din("dpar", [128, DC])
    a_sc = din("a_sc", [128, DC * SPC])         # per-tile A scale column
    selm = din("selm", [128, SPC * 128], BF16)  # 16 selection matrices
    expm = din("expm", [128, SPC * 128], F32R)  # 16 expansion matrices
    identb = din("identb", [128, 128], BF16)

    out_part = nc.dram_tensor("out_part", [L, D_MODEL], F32,
                              kind="ExternalOutput").ap()

    with tile.TileContext(nc) as tc:
        cst = tc.alloc_tile_pool(name="cst", bufs=1)
        dram = tc.alloc_tile_pool(name="dram", bufs=1, space="DRAM")
        pW = tc.alloc_tile_pool(name="pW", bufs=1)

        # ---- constants / weights to SBUF ----
        convd_sb = cst.tile([128, DC * DCONV * 128], BF16)
        nc.sync.dma_start(convd_sb[:], convd[:])
        convb_sb = cst.tile([128, DC], F32)
        nc.sync.dma_start(convb_sb[:], convb[:])
        dtb_sb = cst.tile([128, DC], F32)
        nc.sync.dma_start(dtb_sb[:], dtb[:])
        dpar_sb = cst.tile([128, DC], F32)
        nc.sync.dma_start(dpar_sb[:], dpar[:])
        asc_sb = cst.tile([128, DC * SPC], F32)
        nc.sync.dma_start(asc_sb[:], a_sc[:])
        sel_sb = cst.tile([128, SPC * 128], BF16)
        nc.sync.dma_start(sel_sb[:], selm[:])
        exp_sb = cst.tile([128, SPC * 128], F32R)
        nc.sync.dma_start(exp_sb[:], expm[:])
        id_sb = cst.tile([128, 128], BF16)
        nc.sync.dma_start(id_sb[:], identb[:])
        eps_sb = cst.tile([128, 1], F32)
        nc.vector.memset(eps_sb[:], EPS)
        wx_sb = [cst.tile([128, 96], BF16, tag=f"wx{d}", name=f"wx{d}")
                 for d in range(DC)]
        for d in range(DC):
            nc.sync.dma_start(wx_sb[d][:], wxT[128 * d:128 * (d + 1), :])
        wdt_sb = cst.tile([DT_RANK, DLOC], BF16)
        nc.sync.dma_start(wdt_sb[:], wdtT[:])
        wout_sb = [cst.tile([128, D_MODEL], BF16, tag=f"wo{d}", name=f"wo{d}")
                   for d in range(DC)]
        for d in range(DC):
            nc.sync.dma_start(wout_sb[d][:], woutT[128 * d:128 * (d + 1), :])
        winx_sb = [pW.tile([128, DLOC], BF16, tag=f"winx{k}", name=f"winx{k}")
                   for k in range(KC)]
        winz_sb = [pW.tile([128, DLOC], BF16, tag=f"winz{k}", name=f"winz{k}")
                   for k in range(KC)]
        hT_all = pW.tile([128, KC * L], BF16)
        hT_v = hT_all[:].rearrange("p (k t) -> p k t", k=KC)

        # ====== Phases A+B interleaved: RMSNorm/transpose windows feed
        # in_proj x windows; conv on PE; x_proj -> AllReduce; z under AR ====
        for k in range(KC):
            nc.sync.dma_start(winx_sb[k][:], winx[128 * k:128 * (k + 1), :])
            nc.sync.dma_start(winz_sb[k][:], winz[128 * k:128 * (k + 1), :])

        pBC = tc.alloc_tile_pool(name="pBC", bufs=1, side="right")
        zg = [pBC.tile([128, L], BF16, tag=f"zg{d}", name=f"zg{d}")
              for d in range(DC)]
        xb = [pBC.tile([128, L], BF16, tag=f"xb{d}", name=f"xb{d}")
              for d in range(DC)]
        pDE = tc.alloc_tile_pool(name="pDE", bufs=1, side="right")
        xdbl_p = pDE.tile([96, L], BF16)
        xdbl_sb = pDE.tile([96, L], BF16)
        bbc = pDE.tile([128, L], BF16)
        cbc = pDE.tile([128, L], BF16)
        pX = tc.alloc_tile_pool(name="pX", bufs=1, side="right")
        xpad = [pX.tile([128, L + DCONV], BF16, tag=f"xpad{d}",
                        name=f"xpad{d}") for d in range(DC)]
        for d in range(DC):
            nc.vector.memset(xpad[d][:, 0:DCONV - 1], 0.0)

        ps_fr = tc.alloc_tile_pool(name="ps_fr", bufs=1, space="PSUM")
        WN = L // 512   # 4 in_proj windows of 512 timesteps

        with tc.tile_pool(name="pA", bufs=4) as pA, \
             tc.tile_pool(name="pA2", bufs=3) as pA2:

            def emit_rt(rt):
                t0 = 128 * rt
                ld = pA.tile([128, D_MODEL], BF16, tag="ld")
                nc.sync.dma_start(ld[:], rin[t0:t0 + 128, :])
                sq = pA2.tile([128, D_MODEL], BF16, tag="sq", bufs=1)
                st = pA2.tile([128, 1], F32, tag="st")
                nc.scalar.activation(sq[:], ld[:], AF.Square, accum_out=st[:])
                sg = pA2.tile([128, 1], F32, tag="sg")
                nc.scalar.activation(sg[:], st[:], AF.Sqrt,
                                     bias=eps_sb[:], scale=1.0 / D_MODEL)
                rstd = pA2.tile([128, 1], F32, tag="rstd")
                nc.vector.reciprocal(rstd[:], sg[:])
                hrow = pA2.tile([128, D_MODEL], BF16, tag="hrow")
                nc.vector.tensor_scalar_mul(hrow[:], ld[:], rstd[:])
                for c in range(2):
                    pt = ps_fr.tile([128, 512], BF16, tag="tr", bufs=2,
                                    name="pt")
                    for j in range(4):
                        k = 4 * c + j
                        nc.tensor.transpose(pt[:, 128 * j:128 * (j + 1)],
                                            hrow[:, 128 * k:128 * (k + 1)],
                                            id_sb[:])
                    dst = hT_v[:, 4 * c:4 * (c + 1), t0:t0 + 128]
                    psrc = pt[:].rearrange("p (k t) -> p k t", k=4)
                    nc.vector.tensor_copy(dst, psrc)

            def emit_proj_window(w_sb, w, dest_fn):
                for d in range(DC):
                    pm = ps_fr.tile([128, 512], F32, tag=f"px{d}",
                                    name="pm")
                    for k in range(KC):
                        nc.tensor.matmul(
                            pm[:], w_sb[k][:, 128 * d:128 * (d + 1)],
                            hT_v[:, k, 512 * w:512 * (w + 1)],
                            start=(k == 0), stop=(k == KC - 1))
                    dest_fn(d, w, pm)

            def evict_x(d, w, pm):
                o = DCONV - 1 + 512 * w
                nc.scalar.activation(xpad[d][:, o:o + 512], pm[:], AF.Copy)

            for w in range(WN):
                for rt in range(4 * w, 4 * w + 4):
                    emit_rt(rt)
                emit_proj_window(winx_sb, w, evict_x)

        # conv: 4 shifted diag matmuls, k-outer over 4 window psums per d
        for d in range(DC):
            pms = [ps_fr.tile([128, 512], F32, tag=f"px{w}", name="pm")
                   for w in range(WN)]
            for k in range(DCONV):
                for w in range(WN):
                    o = 512 * w + k
                    nc.tensor.matmul(
                        pms[w][:],
                        convd_sb[:, 128 * (DCONV * d + k):
                                 128 * (DCONV * d + k + 1)],
                        xpad[d][:, o:o + 512],
                        start=(k == 0), stop=(k == DCONV - 1))
            for w in range(WN):
                nc.scalar.activation(xb[d][:, 512 * w:512 * (w + 1)],
                                     pms[w][:], AF.Silu,
                                     bias=convb_sb[:, d:d + 1])
        pX.release()

        # x_proj partials
        for w in range(WN):
            pm = ps_fr.tile([128, 512], F32, tag="px0", name="pm")
            for d in range(DC):
                nc.tensor.matmul(pm[0:96, :], wx_sb[d][:],
                                 xb[d][:, 512 * w:512 * (w + 1)],
                                 start=(d == 0), stop=(d == DC - 1))
            nc.scalar.activation(xdbl_p[:, 512 * w:512 * (w + 1)],
                                 pm[0:96, :], AF.Copy)

        # ====== Phase D: AllReduce (bf16); z-half fills the latency ======
        # bounce path on the DVE DMA queue so it isn't stuck behind SP's
        # paced rin loads
        bounce_i = dram.tile([96, L], BF16)
        bounce_o = dram.tile([96, L], BF16)
        nc.scalar.dma_start(bounce_i[:], xdbl_p[:])
        nc.gpsimd.collective_compute(
            "AllReduce", OP.add,
            replica_groups=[[0, 1, 2, 3], [4, 5, 6, 7]],
            ins=[bounce_i.opt()], outs=[bounce_o.opt()])
        nc.scalar.dma_start(xdbl_sb[:], bounce_o[:])

        def evict_z(d, w, pm):
            nc.scalar.activation(zg[d][:, 512 * w:512 * (w + 1)], pm[:],
                                 AF.Silu)

        with tc.tile_pool(name="pZ", bufs=1) as _pz:
            for w in range(WN):
                for d in range(DC):
                    pm = ps_fr.tile([128, 512], F32, tag=f"px{d}", name="pm")
                    for k in range(KC):
                        nc.tensor.matmul(
                            pm[:], winz_sb[k][:, 128 * d:128 * (d + 1)],
                            hT_v[:, k, 512 * w:512 * (w + 1)],
                            start=(k == 0), stop=(k == KC - 1))
                    evict_z(d, w, pm)

        dtlow = xdbl_sb[0:64, :]
        for i in range(NSUB):
            nc.scalar.dma_start(bbc[NST * i:NST * (i + 1), :],
                                xdbl_sb[64:80, :])
            nc.scalar.dma_start(cbc[NST * i:NST * (i + 1), :],
                                xdbl_sb[80:96, :])
        pW.release()
        ps_fr.release()

        # ====== Phase F: dt path + selective scan (fused per d) ======
        pY = tc.alloc_tile_pool(name="pY", bufs=1, side="right")
        yg = [pY.tile([128, L], BF16, tag=f"yg{d}", name=f"yg{d}")
              for d in range(DC)]
        with tc.tile_pool(name="pF", bufs=2) as pF, \
             tc.tile_pool(name="pQ", bufs=2) as pQ, \
             tc.tile_pool(name="ps_y", bufs=1, space="PSUM") as ps_y, \
             tc.tile_pool(name="ps_f", bufs=2, space="PSUM") as ps_f:
            dt_ds = {}
            ub_ds = {}

            def emit_prep(d):
                # dt = softplus(dt_proj(dtlow)+bias) = Ln(1 + Exp(.+bias))
                u_t = pF.tile([128, L], F32, tag="u_t", bufs=1, name="u_t")
                for t2 in range(2):
                    pm = ps_f.tile([128, 1024], F32, tag="pa", name="pa")
                    for h in range(2):
                        o = 1024 * t2 + 512 * h
                        nc.tensor.matmul(
                            pm[:, 512 * h:512 * (h + 1)],
                            wdt_sb[:, 128 * d:128 * (d + 1)],
                            dtlow[:, o:o + 512], start=True, stop=True,
                            skip_group_check=True)
                    nc.scalar.activation(u_t[:, 1024 * t2:1024 * (t2 + 1)],
                                         pm[:], AF.Exp,
                                         bias=dtb_sb[:, d:d + 1])
                dt_d = pF.tile([128, L], F32R, tag="dt_d", bufs=2,
                               name="dt_d")
                nc.scalar.activation(dt_d[:], u_t[:], AF.Ln, bias=1.0)
                ub_d = pF.tile([128, L], BF16, tag="ub_d", bufs=2, name="ub_d")
                nc.vector.tensor_mul(ub_d[:], dt_d[:].bitcast(F32), xb[d][:])
                ub_sc = dram.tile([128, L], BF16, tag="ub_sc", bufs=2,
                                  name="ub_sc")
                nc.sync.dma_start(ub_sc[:], ub_d[:])
                dt_ds[d] = dt_d
                ub_ds[d] = ub_sc

            emit_prep(0)

            NG = DC * SPC  # 64 global subtiles
            quads = {}
            a_ts = {}
            h_ts = {}
            hc_ts = {}
            ypsums = {}

            def emit_quad_reads(qg):
                # quad qg covers subtiles 4qg..4qg+3 of d = qg//4
                quad = pQ.tile([128, 4 * L], BF16, tag="quad", bufs=2)
                quad_v = quad[:].rearrange(
                    "(i n) (s t) -> i n s t", i=NSUB, s=4)
                src_r = ub_ds[qg // 4][32 * (qg % 4):32 * (qg % 4 + 1),
                                       :].rearrange("(s i) t -> i s t", s=4)
                for n in range(NST):
                    nc.sync.dma_start(quad_v[:, n, :, :], src_r)
                quads[qg] = quad

            def emit_S0(g):
                # a = exp(dt*A): PE replicate-matmul + ACT Exp
                d, sidx = divmod(g, SPC)
                dt_d = dt_ds[d]
                a_t = pF.tile([128, L], F32, tag="a", bufs=2)
                for t2 in range(2):
                    pm = ps_f.tile([128, 1024], F32, tag="pa")
                    for h in range(2):
                        o = 1024 * t2 + 512 * h
                        nc.tensor.matmul(
                            pm[:, 512 * h:512 * (h + 1)],
                            exp_sb[:, 128 * sidx:128 * (sidx + 1)],
                            dt_d[:, o:o + 512], start=True, stop=True,
                            skip_group_check=True)
                    nc.scalar.activation(
                        a_t[:, 1024 * t2:1024 * (t2 + 1)], pm[:], AF.Exp,
                        scale=asc_sb[:, d * SPC + sidx:d * SPC + sidx + 1])
                a_ts[g] = a_t

            def emit_S12(g):
                # b = ub_exp*B (DVE), h = scan (DVE), hc = h*C (Pool/DVE)
                sq = g % 4
                quad = quads[g // 4]
                b_t = pF.tile([128, L], BF16, tag="b")
                nc.vector.tensor_mul(b_t[:], quad[:, L * sq:L * (sq + 1)],
                                     bbc[:])
                h_t = pF.tile([128, L], BF16, tag="h")
                nc.vector.tensor_tensor_scan(h_t[:], a_ts.pop(g)[:], b_t[:],
                                             0.0, OP.mult, OP.add)
                hc = pF.tile([128, L], BF16, tag="hc")
                if g % 9 == 4:
                    nc.vector.tensor_mul(hc[:], h_t[:], cbc[:])
                else:
                    nc.gpsimd.tensor_mul(hc[:], h_t[:], cbc[:])
                hc_ts[g] = hc

            def emit_S4(g):
                d, sidx = divmod(g, SPC)
                if sidx == 0:
                    ypsums[d] = ps_y.tile([128, L], F32, tag="ypsum",
                                          name="ypsum")
                hc = hc_ts.pop(g)
                for tq in range(4):
                    nc.tensor.matmul(
                        ypsums[d][:, 512 * tq:512 * (tq + 1)],
                        sel_sb[:, 128 * sidx:128 * (sidx + 1)],
                        hc[:, 512 * tq:512 * (tq + 1)],
                        start=(sidx == 0), stop=(sidx == SPC - 1),
                        skip_group_check=True)
                if sidx == SPC - 1:
                    # y2 = D*x + y ;  yg = y2 * silu(z)
                    for hh in range(2):
                        o = 1024 * hh
                        y2 = pF.tile([128, 1024], F32, tag="y2", bufs=2)
                        nc.vector.scalar_tensor_tensor(
                            y2[:], xb[d][:, o:o + 1024], dpar_sb[:, d:d + 1],
                            ypsums[d][:, o:o + 1024], OP.mult, OP.add)
                        nc.gpsimd.tensor_mul(yg[d][:, o:o + 1024], y2[:],
                                             zg[d][:, o:o + 1024])

            emit_quad_reads(0)
            for step in range(NG + 2):
                g0, g1, g2 = step, step - 1, step - 2
                if g0 < NG:
                    if g0 % SPC == 8 and g0 // SPC + 1 < DC:
                        emit_prep(g0 // SPC + 1)
                    gq = g0 + 2
                    if gq % 4 == 0 and gq < NG:
                        emit_quad_reads(gq // 4)
                    emit_S0(g0)
                if 0 <= g1 < NG:
                    emit_S12(g1)
                if 0 <= g2 < NG:
                    emit_S4(g2)
        # ====== Phase G: out_proj ======
        with tc.tile_pool(name="pG", bufs=3) as pG, \
             tc.tile_pool(name="ps_g", bufs=2, space="PSUM") as ps_g:
            for tb in range(RT):
                pm = ps_g.tile([128, D_MODEL], F32, tag="pmG")
                for d in range(DC):
                    for h in range(2):
                        nc.tensor.matmul(
                            pm[:, 512 * h:512 * (h + 1)],
                            yg[d][:, 128 * tb:128 * (tb + 1)],
                            wout_sb[d][:, 512 * h:512 * (h + 1)],
                            start=(d == 0), stop=(d == DC - 1))
                osb = pG.tile([128, D_MODEL], F32, tag="osb")
                nc.scalar.activation(osb[:], pm[:], AF.Copy)
                nc.sync.dma_start(out_part[128 * tb:128 * (tb + 1), :],
                                  osb[:])
        pY.release()
        pDE.release()
        pBC.release()
        cst.release()
        dram.release()
    nc.compile()

    return nc


_NC_CACHE = None


def _get_nc():
    global _NC_CACHE
    if _NC_CACHE is None:
        _NC_CACHE = _build()
    return _NC_CACHE


def kernel(input_ids=None, hidden_states=None, residual=None, norm_w=None,
           in_proj_w=None, conv_w=None, conv_b=None, x_proj_w=None,
           dt_proj_w=None, dt_proj_b=None, A_log=None, D_param=None,
           out_proj_w=None, **kwargs):
    import ml_dtypes
    bf16 = np.dtype(ml_dtypes.bfloat16)

    hs = np.asarray(hidden_states, np.float32)
    rs = np.asarray(residual, np.float32)
    ipw = np.asarray(in_proj_w, np.float32)
    cw = np.asarray(conv_w, np.float32)
    cb = np.asarray(conv_b, np.float32)
    xpw = np.asarray(x_proj_w, np.float32)
    dpw = np.asarray(dt_proj_w, np.float32)
    dpb = np.asarray(dt_proj_b, np.float32)
    al = np.asarray(A_log, np.float32)
    dpr = np.asarray(D_param, np.float32)
    opw = np.asarray(out_proj_w, np.float32)
    nw = np.asarray(norm_w, np.float32)

    r_full = hs + rs                               # host-side residual add

    def colpack(v):  # [DLOC] -> [128, DC], col d = v[d*128:(d+1)*128]
        return np.ascontiguousarray(v.reshape(DC, 128).T).astype(np.float32)

    selm = np.zeros((128, SPC * 128), np.float32)
    expm = np.zeros((128, SPC * 128), np.float32)
    for s in range(SPC):
        for i in range(NSUB):
            m = s * NSUB + i
            for n in range(NST):
                p = i * NST + n
                selm[p, s * 128 + m] = 1.0
                expm[m, s * 128 + p] = 1.0
    identb = np.eye(128, dtype=np.float32)

    nc = _get_nc()
    in_maps = []
    for c in range(N_CORES):
        b, k = c // TPG, c % TPG
        sl = slice(k * DLOC, (k + 1) * DLOC)
        slz = slice(D_INNER + k * DLOC, D_INNER + (k + 1) * DLOC)

        conv4 = cw[sl, 0, :]                       # [DLOC, 4]
        convd = np.zeros((128, DC * DCONV * 128), np.float32)
        for d in range(DC):
            for kk in range(DCONV):
                blk = DCONV * d + kk
                np.fill_diagonal(
                    convd[:, 128 * blk:128 * (blk + 1)],
                    conv4[128 * d:128 * (d + 1), kk])

        A = -np.exp(al[sl])                        # [DLOC, 16]
        a_sc = np.zeros((128, DC * SPC), np.float32)
        for d in range(DC):
            for s in range(SPC):
                rows = A[d * 128 + s * NSUB: d * 128 + (s + 1) * NSUB, :]
                a_sc[:, d * SPC + s] = rows.reshape(128)

        in_maps.append(dict(
            rin=r_full[b].astype(bf16),
            winx=np.ascontiguousarray(ipw[sl].T * nw[:, None]).astype(bf16),
            winz=np.ascontiguousarray(ipw[slz].T * nw[:, None]).astype(bf16),
            wxT=np.ascontiguousarray(xpw[:, sl].T).astype(bf16),
            wdtT=np.ascontiguousarray(dpw[sl].T).astype(bf16),
            woutT=np.ascontiguousarray(opw[:, sl].T).astype(bf16),
            convd=convd.astype(bf16),
            convb=colpack(cb[sl]),
            dtb=colpack(dpb[sl]),
            dpar=colpack(dpr[sl]),
            a_sc=a_sc,
            selm=selm.astype(bf16),
            expm=expm,
            identb=identb.astype(bf16),
        ))

    res = run_bass_kernel_spmd(nc, in_maps, core_ids=list(range(N_CORES)))
    outs = [res.results[c]["out_part"] for c in range(N_CORES)]
    full = np.stack([
        sum(outs[b * TPG + k] for k in range(TPG)) for b in range(BATCH)
    ]).astype(np.float32)
    return full


# revision 29
# speedup vs baseline: 1.2661x; 1.0167x over previous
"""Mamba block kernel for Trainium2 (8 NeuronCores).

Sharding: batch (2-way) x tensor-parallel over d_inner (4-way).
Core c handles batch c//4 and d_inner channels [(c%4)*512, (c%4+1)*512).
Host folds norm_w into in_proj, pre-adds hidden+residual (bf16), and sums
the 4 TP partial outputs per batch.

Device pipeline per core (one NEFF, phases overlap via Tile scheduling):
  A. RMSNorm of r=hid+res (ACT square-accumulate variance), PE-transpose
     via bf16 identity matmuls -> hT_all [1024, L] bf16 (k-major columns)
  B. in_proj x-half (k-outer bf16 matmuls, 1024-wide moving), causal
     depthwise conv as 4 shifted diag-matmuls on PE accumulating in PSUM
     (SiLU fused in the ACT eviction), x_proj partials
  D. AllReduce of bf16 x_dbl partials (groups [[0-3],[4-7]], DRAM bounce);
     the z-half of in_proj + SiLU runs under the collective's latency
  F. per d-chunk (software-pipelined):
       dt = softplus via ACT: u = Exp(dt_proj + bias), dt = Ln(u + 1)
       ub = dt*x -> 4 DRAM copies -> per-quad expansion DMAs (4 per quad)
       selective scan over 16 expanded tiles [128=(8 d x 16 n), L]:
         a = exp(dt*A): PE f32r replicate-matmul, ACT Exp w/ A scale
         b = ub_exp*B broadcast (DVE bf16 2x)
         h = tensor_tensor_scan on DVE (1x, the hard floor)
         hc = h*C (split DVE / GpSimd Pool to balance engine load)
         y = sel 0/1-matmuls accumulating 16 subtiles into one PSUM tile
       y2 = D*x + y (DVE stt), yg = y2*silu(z) (Pool)
  G. out_proj partial (yg-block stationary, wout moving) -> [L, 1024] f32
"""

import sys

sys.path.insert(0, "/opt/trn_rl_repo")

import numpy as np

import concourse.bacc as bacc
import concourse.tile as tile
from concourse import mybir
from concourse.bass_utils import run_bass_kernel_spmd

F32 = mybir.dt.float32
F32R = mybir.dt.float32r
BF16 = mybir.dt.bfloat16
AF = mybir.ActivationFunctionType
OP = mybir.AluOpType

D_MODEL = 1024
D_INNER = 2048
NST = 16          # d_state
DT_RANK = 64
DCONV = 4
BATCH = 2
L = 2048
EPS = 1e-5

N_CORES = 8
TPG = 4                    # tensor-parallel group size
DLOC = D_INNER // TPG      # 512 channels per core
DC = DLOC // 128           # 4 partition chunks of x-channels
KC = D_MODEL // 128        # 8 contraction chunks
RT = L // 128              # 16 row tiles
NSUB = 128 // NST          # 8 d-channels per expanded tile
SPC = 128 // NSUB          # 16 subtiles per d-chunk

# hc-mul engine assignment: subtile (d*SPC+s) goes to Pool unless in this set
HC_DVE = frozenset()


def _build():
    nc = bacc.Bacc("TRN2", target_bir_lowering=False, debug=False,
                   enable_asserts=True, num_devices=N_CORES)

    def din(name, shape, dt=F32):
        return nc.dram_tensor(name, shape, dt, kind="ExternalInput").ap()

    rin = din("rin", [L, D_MODEL], BF16)        # hid+res, host-added
    winx = din("winx", [D_MODEL, DLOC], BF16)   # in_proj_w[x-slice].T * nw
    winz = din("winz", [D_MODEL, DLOC], BF16)   # in_proj_w[z-slice].T * nw
    wxT = din("wxT", [DLOC, 96], BF16)          # x_proj_w[:, slice].T
    wdtT = din("wdtT", [DT_RANK, DLOC], BF16)   # dt_proj_w[slice].T
    woutT = din("woutT", [DLOC, D_MODEL], BF16)  # out_proj_w[:, slice].T
    convd = din("convd", [128, DC * DCONV * 128], BF16)  # diag stationaries
    convb = din("convb", [128, DC])
    dtb = din("dtb", [128, DC])
    dpar = din("dpar", [128, DC])
    a_sc = din("a_sc", [128, DC * SPC])         # per-tile A scale column
    selm = din("selm", [128, SPC * 128], BF16)  # 16 selection matrices
    expm = din("expm", [128, SPC * 128], F32R)  # 16 expansion matrices
    identb = din("identb", [128, 128], BF16)

    out_part = nc.dram_tensor("out_part", [L, D_MODEL], F32,
                              kind="ExternalOutput").ap()

    with tile.TileContext(nc) as tc:
        cst = tc.alloc_tile_pool(name="cst", bufs=1)
        dram = tc.alloc_tile_pool(name="dram", bufs=1, space="DRAM")
        pW = tc.alloc_tile_pool(name="pW", bufs=1)

        # ---- constants / weights to SBUF ----
        convd_sb = cst.tile([128, DC * DCONV * 128], BF16)
        nc.sync.dma_start(convd_sb[:], convd[:])
        convb_sb = cst.tile([128, DC], F32)
        nc.sync.dma_start(convb_sb[:], convb[:])
        dtb_sb = cst.tile([128, DC], F32)
        nc.sync.dma_start(dtb_sb[:], dtb[:])
        dpar_sb = cst.tile([128, DC], F32)
        nc.sync.dma_start(dpar_sb[:], dpar[:])
        asc_sb = cst.tile([128, DC * SPC], F32)
        nc.sync.dma_start(asc_sb[:], a_sc[:])
        sel_sb = cst.tile([128, SPC * 128], BF16)
        nc.sync.dma_start(sel_sb[:], selm[:])
        exp_sb = cst.tile([128, SPC * 128], F32R)
        nc.sync.dma_start(exp_sb[:], expm[:])
        id_sb = cst.tile([128, 128], BF16)
        nc.sync.dma_start(id_sb[:], identb[:])
        eps_sb = cst.tile([128, 1], F32)
        nc.vector.memset(eps_sb[:], EPS)
        wx_sb = [cst.tile([128, 96], BF16, tag=f"wx{d}", name=f"wx{d}")
                 for d in range(DC)]
        for d in range(DC):
            nc.sync.dma_start(wx_sb[d][:], wxT[128 * d:128 * (d + 1), :])
        wdt_sb = cst.tile([DT_RANK, DLOC], BF16)
        nc.sync.dma_start(wdt_sb[:], wdtT[:])
        wout_sb = [cst.tile([128, D_MODEL], BF16, tag=f"wo{d}", name=f"wo{d}")
                   for d in range(DC)]
        for d in range(DC):
            nc.sync.dma_start(wout_sb[d][:], woutT[128 * d:128 * (d + 1), :])
        winx_sb = [pW.tile([128, DLOC], BF16, tag=f"winx{k}", name=f"winx{k}")
                   for k in range(KC)]
        winz_sb = [pW.tile([128, DLOC], BF16, tag=f"winz{k}", name=f"winz{k}")
                   for k in range(KC)]
        hT_all = pW.tile([128, KC * L], BF16)
        hT_v = hT_all[:].rearrange("p (k t) -> p k t", k=KC)

        # ====== Phases A+B interleaved: RMSNorm/transpose windows feed
        # in_proj x windows; conv on PE; x_proj -> AllReduce; z under AR ====
        for k in range(KC):
            nc.sync.dma_start(winx_sb[k][:], winx[128 * k:128 * (k + 1), :])
            nc.sync.dma_start(winz_sb[k][:], winz[128 * k:128 * (k + 1), :])

        pBC = tc.alloc_tile_pool(name="pBC", bufs=1, side="right")
        zg = [pBC.tile([128, L], BF16, tag=f"zg{d}", name=f"zg{d}")
              for d in range(DC)]
        xb = [pBC.tile([128, L], BF16, tag=f"xb{d}", name=f"xb{d}")
              for d in range(DC)]
        pDE = tc.alloc_tile_pool(name="pDE", bufs=1, side="right")
        xdbl_p = pDE.tile([96, L], BF16)
        xdbl_sb = pDE.tile([96, L], BF16)
        bbc = pDE.tile([128, L], BF16)
        cbc = pDE.tile([128, L], BF16)
        pX = tc.alloc_tile_pool(name="pX", bufs=1, side="right")
        xpad = [pX.tile([128, L + DCONV], BF16, tag=f"xpad{d}",
                        name=f"xpad{d}") for d in range(DC)]
        for d in range(DC):
            nc.vector.memset(xpad[d][:, 0:DCONV - 1], 0.0)

        ps_fr = tc.alloc_tile_pool(name="ps_fr", bufs=1, space="PSUM")
        WN = L // 512   # 4 in_proj windows of 512 timesteps

        with tc.tile_pool(name="pA", bufs=4) as pA, \
             tc.tile_pool(name="pA2", bufs=3) as pA2:

            def emit_rt(rt):
                t0 = 128 * rt
                ld = pA.tile([128, D_MODEL], BF16, tag="ld")
                nc.scalar.dma_start(ld[:], rin[t0:t0 + 128, :])
                sq = pA2.tile([128, D_MODEL], BF16, tag="sq", bufs=1)
                st = pA2.tile([128, 1], F32, tag="st")
                nc.scalar.activation(sq[:], ld[:], AF.Square, accum_out=st[:])
                sg = pA2.tile([128, 1], F32, tag="sg")
                nc.scalar.activation(sg[:], st[:], AF.Sqrt,
                                     bias=eps_sb[:], scale=1.0 / D_MODEL)
                rstd = pA2.tile([128, 1], F32, tag="rstd")
                nc.vector.reciprocal(rstd[:], sg[:])
                hrow = pA2.tile([128, D_MODEL], BF16, tag="hrow")
                nc.vector.tensor_scalar_mul(hrow[:], ld[:], rstd[:])
                for c in range(2):
                    pt = ps_fr.tile([128, 512], BF16, tag="tr", bufs=2,
                                    name="pt")
                    for j in range(4):
                        k = 4 * c + j
                        nc.tensor.transpose(pt[:, 128 * j:128 * (j + 1)],
                                            hrow[:, 128 * k:128 * (k + 1)],
                                            id_sb[:])
                    dst = hT_v[:, 4 * c:4 * (c + 1), t0:t0 + 128]
                    psrc = pt[:].rearrange("p (k t) -> p k t", k=4)
                    nc.vector.tensor_copy(dst, psrc)

            def emit_proj_window(w_sb, w, dest_fn):
                for d in range(DC):
                    pm = ps_fr.tile([128, 512], F32, tag=f"px{d}",
                                    name="pm")
                    for k in range(KC):
                        nc.tensor.matmul(
                            pm[:], w_sb[k][:, 128 * d:128 * (d + 1)],
                            hT_v[:, k, 512 * w:512 * (w + 1)],
                            start=(k == 0), stop=(k == KC - 1))
                    dest_fn(d, w, pm)

            def evict_x(d, w, pm):
                o = DCONV - 1 + 512 * w
                nc.vector.tensor_copy(xpad[d][:, o:o + 512], pm[:])

            def emit_conv_w(d, w):
                # causal conv window: xpad cols [512w, 512w+512+3) suffice
                pm = ps_fr.tile([128, 512], F32, tag="pxp", bufs=2,
                                name="pm")
                for k in range(DCONV):
                    o = 512 * w + k
                    nc.tensor.matmul(
                        pm[:],
                        convd_sb[:, 128 * (DCONV * d + k):
                                 128 * (DCONV * d + k + 1)],
                        xpad[d][:, o:o + 512],
                        start=(k == 0), stop=(k == DCONV - 1))
                nc.scalar.activation(xb[d][:, 512 * w:512 * (w + 1)],
                                     pm[:], AF.Silu,
                                     bias=convb_sb[:, d:d + 1])

            def emit_xproj_w(w):
                pm = ps_fr.tile([128, 512], F32, tag="pxp", bufs=2,
                                name="pm")
                for d in range(DC):
                    nc.tensor.matmul(pm[0:96, :], wx_sb[d][:],
                                     xb[d][:, 512 * w:512 * (w + 1)],
                                     start=(d == 0), stop=(d == DC - 1))
                nc.vector.tensor_copy(xdbl_p[:, 512 * w:512 * (w + 1)],
                                      pm[0:96, :])

            for w in range(WN):
                for rt in range(4 * w, 4 * w + 4):
                    emit_rt(rt)
                emit_proj_window(winx_sb, w, evict_x)
                for d in range(DC):
                    emit_conv_w(d, w)
                emit_xproj_w(w)
        pX.release()

        # ====== Phase D: AllReduce (bf16); z-half fills the latency ======
        # bounce path on the DVE DMA queue so it isn't stuck behind SP's
        # paced rin loads
        bounce_i = dram.tile([96, L], BF16)
        bounce_o = dram.tile([96, L], BF16)
        nc.gpsimd.dma_start(bounce_i[:], xdbl_p[:])
        nc.gpsimd.collective_compute(
            "AllReduce", OP.add,
            replica_groups=[[0, 1, 2, 3], [4, 5, 6, 7]],
            ins=[bounce_i.opt()], outs=[bounce_o.opt()])
        nc.gpsimd.dma_start(xdbl_sb[:], bounce_o[:])

        def evict_z(d, w, pm):
            nc.scalar.activation(zg[d][:, 512 * w:512 * (w + 1)], pm[:],
                                 AF.Silu)

        with tc.tile_pool(name="pZ", bufs=1) as _pz:
            for w in range(WN):
                for d in range(DC):
                    pm = ps_fr.tile([128, 512], F32, tag=f"px{d}", name="pm")
                    for k in range(KC):
                        nc.tensor.matmul(
                            pm[:], winz_sb[k][:, 128 * d:128 * (d + 1)],
                            hT_v[:, k, 512 * w:512 * (w + 1)],
                            start=(k == 0), stop=(k == KC - 1))
                    evict_z(d, w, pm)

        dtlow = xdbl_sb[0:64, :]
        for i in range(NSUB):
            nc.gpsimd.dma_start(bbc[NST * i:NST * (i + 1), :],
                                xdbl_sb[64:80, :])
            nc.gpsimd.dma_start(cbc[NST * i:NST * (i + 1), :],
                                xdbl_sb[80:96, :])
        pW.release()
        ps_fr.release()

        # ====== Phase F: dt path + selective scan (fused per d) ======
        pY = tc.alloc_tile_pool(name="pY", bufs=1, side="right")
        yg = [pY.tile([128, L], BF16, tag=f"yg{d}", name=f"yg{d}")
              for d in range(DC)]
        with tc.tile_pool(name="pF", bufs=2) as pF, \
             tc.tile_pool(name="pQ", bufs=2) as pQ, \
             tc.tile_pool(name="ps_y", bufs=1, space="PSUM") as ps_y, \
             tc.tile_pool(name="ps_f", bufs=2, space="PSUM") as ps_f:
            dt_ds = {}
            ub_ds = {}

            def emit_prep(d):
                # dt = softplus(dt_proj(dtlow)+bias) = Ln(1 + Exp(.+bias))
                u_t = pF.tile([128, L], F32, tag="u_t", bufs=1, name="u_t")
                for t2 in range(2):
                    pm = ps_f.tile([128, 1024], F32, tag="pa", name="pa")
                    for h in range(2):
                        o = 1024 * t2 + 512 * h
                        nc.tensor.matmul(
                            pm[:, 512 * h:512 * (h + 1)],
                            wdt_sb[:, 128 * d:128 * (d + 1)],
                            dtlow[:, o:o + 512], start=True, stop=True,
                            skip_group_check=True)
                    nc.scalar.activation(u_t[:, 1024 * t2:1024 * (t2 + 1)],
                                         pm[:], AF.Exp,
                                         bias=dtb_sb[:, d:d + 1])
                dt_d = pF.tile([128, L], F32R, tag="dt_d", bufs=2,
                               name="dt_d")
                nc.scalar.activation(dt_d[:], u_t[:], AF.Ln, bias=1.0)
                ub_d = pF.tile([128, L], BF16, tag="ub_d", bufs=2, name="ub_d")
                nc.vector.tensor_mul(ub_d[:], dt_d[:].bitcast(F32), xb[d][:])
                ub_sc = dram.tile([128, L], BF16, tag="ub_sc", bufs=2,
                                  name="ub_sc")
                nc.sync.dma_start(ub_sc[:], ub_d[:])
                dt_ds[d] = dt_d
                ub_ds[d] = ub_sc

            emit_prep(0)

            NG = DC * SPC  # 64 global subtiles
            quads = {}
            a_ts = {}
            h_ts = {}
            hc_ts = {}
            ypsums = {}

            def emit_quad_reads(qg):
                # quad qg covers subtiles 4qg..4qg+3 of d = qg//4
                quad = pQ.tile([128, 4 * L], BF16, tag="quad", bufs=2)
                quad_v = quad[:].rearrange(
                    "(i n) (s t) -> i n s t", i=NSUB, s=4)
                src_r = ub_ds[qg // 4][32 * (qg % 4):32 * (qg % 4 + 1),
                                       :].rearrange("(s i) t -> i s t", s=4)
                for n in range(NST):
                    nc.sync.dma_start(quad_v[:, n, :, :], src_r)
                quads[qg] = quad

            def emit_S0(g):
                # a = exp(dt*A): PE replicate-matmul + ACT Exp
                d, sidx = divmod(g, SPC)
                dt_d = dt_ds[d]
                a_t = pF.tile([128, L], F32, tag="a", bufs=2)
                for t2 in range(2):
                    pm = ps_f.tile([128, 1024], F32, tag="pa")
                    for h in range(2):
                        o = 1024 * t2 + 512 * h
                        nc.tensor.matmul(
                            pm[:, 512 * h:512 * (h + 1)],
                            exp_sb[:, 128 * sidx:128 * (sidx + 1)],
                            dt_d[:, o:o + 512], start=True, stop=True,
                            skip_group_check=True)
                    nc.scalar.activation(
                        a_t[:, 1024 * t2:1024 * (t2 + 1)], pm[:], AF.Exp,
                        scale=asc_sb[:, d * SPC + sidx:d * SPC + sidx + 1])
                a_ts[g] = a_t

            def emit_S12(g):
                # b = ub_exp*B (DVE), h = scan (DVE), hc = h*C (Pool/DVE)
                sq = g % 4
                quad = quads[g // 4]
                b_t = pF.tile([128, L], BF16, tag="b")
                nc.vector.tensor_mul(b_t[:], quad[:, L * sq:L * (sq + 1)],
                                     bbc[:])
                h_t = pF.tile([128, L], BF16, tag="h")
                nc.vector.tensor_tensor_scan(h_t[:], a_ts.pop(g)[:], b_t[:],
                                             0.0, OP.mult, OP.add)
                hc = pF.tile([128, L], BF16, tag="hc")
                if g % 9 == 4:
                    nc.vector.tensor_mul(hc[:], h_t[:], cbc[:])
                else:
                    nc.gpsimd.tensor_mul(hc[:], h_t[:], cbc[:])
                hc_ts[g] = hc

            def emit_S4(g):
                d, sidx = divmod(g, SPC)
                if sidx == 0:
                    ypsums[d] = ps_y.tile([128, L], F32, tag="ypsum",
                                          name="ypsum")
                hc = hc_ts.pop(g)
                for tq in range(4):
                    nc.tensor.matmul(
                        ypsums[d][:, 512 * tq:512 * (tq + 1)],
                        sel_sb[:, 128 * sidx:128 * (sidx + 1)],
                        hc[:, 512 * tq:512 * (tq + 1)],
                        start=(sidx == 0), stop=(sidx == SPC - 1),
                        skip_group_check=True)
                if sidx == SPC - 1:
                    # y2 = D*x + y ;  yg = y2 * silu(z)
                    for hh in range(2):
                        o = 1024 * hh
                        y2 = pF.tile([128, 1024], F32, tag="y2", bufs=2)
                        nc.vector.scalar_tensor_tensor(
                            y2[:], xb[d][:, o:o + 1024], dpar_sb[:, d:d + 1],
                            ypsums[d][:, o:o + 1024], OP.mult, OP.add)
                        nc.vector.tensor_mul(yg[d][:, o:o + 1024], y2[:],
                                               zg[d][:, o:o + 1024])

            emit_quad_reads(0)
            for step in range(NG + 2):
                g0, g1, g2 = step, step - 1, step - 2
                if g0 < NG:
                    if g0 % SPC == 8 and g0 // SPC + 1 < DC:
                        emit_prep(g0 // SPC + 1)
                    gq = g0 + 2
                    if gq % 4 == 0 and gq < NG:
                        emit_quad_reads(gq // 4)
                    emit_S0(g0)
                if 0 <= g1 < NG:
                    emit_S12(g1)
                if 0 <= g2 < NG:
                    emit_S4(g2)
        # ====== Phase G: out_proj ======
        with tc.tile_pool(name="pG", bufs=3) as pG, \
             tc.tile_pool(name="ps_g", bufs=2, space="PSUM") as ps_g:
            for tb in range(RT):
                pm = ps_g.tile([128, D_MODEL], F32, tag="pmG")
                for d in range(DC):
                    for h in range(2):
                        nc.tensor.matmul(
                            pm[:, 512 * h:512 * (h + 1)],
                            yg[d][:, 128 * tb:128 * (tb + 1)],
                            wout_sb[d][:, 512 * h:512 * (h + 1)],
                            start=(d == 0), stop=(d == DC - 1))
                osb = pG.tile([128, D_MODEL], F32, tag="osb")
                nc.scalar.activation(osb[:], pm[:], AF.Copy)
                nc.sync.dma_start(out_part[128 * tb:128 * (tb + 1), :],
                                  osb[:])
        pY.release()
        pDE.release()
        pBC.release()
        cst.release()
        dram.release()
    nc.compile()

    return nc


_NC_CACHE = None


def _get_nc():
    global _NC_CACHE
    if _NC_CACHE is None:
        _NC_CACHE = _build()
    return _NC_CACHE


def kernel(input_ids=None, hidden_states=None, residual=None, norm_w=None,
           in_proj_w=None, conv_w=None, conv_b=None, x_proj_w=None,
           dt_proj_w=None, dt_proj_b=None, A_log=None, D_param=None,
           out_proj_w=None, **kwargs):
    import ml_dtypes
    bf16 = np.dtype(ml_dtypes.bfloat16)

    hs = np.asarray(hidden_states, np.float32)
    rs = np.asarray(residual, np.float32)
    ipw = np.asarray(in_proj_w, np.float32)
    cw = np.asarray(conv_w, np.float32)
    cb = np.asarray(conv_b, np.float32)
    xpw = np.asarray(x_proj_w, np.float32)
    dpw = np.asarray(dt_proj_w, np.float32)
    dpb = np.asarray(dt_proj_b, np.float32)
    al = np.asarray(A_log, np.float32)
    dpr = np.asarray(D_param, np.float32)
    opw = np.asarray(out_proj_w, np.float32)
    nw = np.asarray(norm_w, np.float32)

    r_full = hs + rs                               # host-side residual add

    def colpack(v):  # [DLOC] -> [128, DC], col d = v[d*128:(d+1)*128]
        return np.ascontiguousarray(v.reshape(DC, 128).T).astype(np.float32)

    selm = np.zeros((128, SPC * 128), np.float32)
    expm = np.zeros((128, SPC * 128), np.float32)
    for s in range(SPC):
        for i in range(NSUB):
            m = s * NSUB + i
            for n in range(NST):
                p = i * NST + n
                selm[p, s * 128 + m] = 1.0
                expm[m, s * 128 + p] = 1.0
    identb = np.eye(128, dtype=np.float32)

    nc = _get_nc()
    in_maps = []
    for c in range(N_CORES):
        b, k = c // TPG, c % TPG
        sl = slice(k * DLOC, (k + 1) * DLOC)
        slz = slice(D_INNER + k * DLOC, D_INNER + (k + 1) * DLOC)

        conv4 = cw[sl, 0, :]                       # [DLOC, 4]
        convd = np.zeros((128, DC * DCONV * 128), np.float32)
        for d in range(DC):
            for kk in range(DCONV):
                blk = DCONV * d + kk
                np.fill_diagonal(
                    convd[:, 128 * blk:128 * (blk + 1)],
                    conv4[128 * d:128 * (d + 1), kk])

        A = -np.exp(al[sl])                        # [DLOC, 16]
        a_sc = np.zeros((128, DC * SPC), np.float32)
        for d in range(DC):
            for s in range(SPC):
                rows = A[d * 128 + s * NSUB: d * 128 + (s + 1) * NSUB, :]
                a_sc[:, d * SPC + s] = rows.reshape(128)

        in_maps.append(dict(
            rin=r_full[b].astype(bf16),
            winx=np.ascontiguousarray(ipw[sl].T * nw[:, None]).astype(bf16),
            winz=np.ascontiguousarray(ipw[slz].T * nw[:, None]).astype(bf16),
            wxT=np.ascontiguousarray(xpw[:, sl].T).astype(bf16),
            wdtT=np.ascontiguousarray(dpw[sl].T).astype(bf16),
            woutT=np.ascontiguousarray(opw[:, sl].T).astype(bf16),
            convd=convd.astype(bf16),
            convb=colpack(cb[sl]),
            dtb=colpack(dpb[sl]),
            dpar=colpack(dpr[sl]),
            a_sc=a_sc,
            selm=selm.astype(bf16),
            expm=expm,
            identb=identb.astype(bf16),
        ))

    res = run_bass_kernel_spmd(nc, in_maps, core_ids=list(range(N_CORES)))
    outs = [res.results[c]["out_part"] for c in range(N_CORES)]
    full = np.stack([
        sum(outs[b * TPG + k] for k in range(TPG)) for b in range(BATCH)
    ]).astype(np.float32)
    return full
